# revision 5
# baseline (speedup 1.0000x reference)
"""3-layer GAT (PyG GATConv-style) on 8 Trainium2 NeuronCores — v2.

Strategy (dst-node sharding, all fp math on device):
  - Nodes are permuted into 100352 slots = 8 shards x 98 blocks x 128 via an
    LPT bin-pack on in-degree so every block has ~equal incoming-edge count
    (cmax = ceil(max_block_edges/128) drops 6 -> 5).
  - Edges (incl. self-loops) are partitioned by dst block; per (core, block)
    the edge list is padded to cmax 128-edge chunks; all 8 cores run one SPMD
    program per stage.
  - Segment softmax + weighted segment-sum run on TensorE via HOST-BUILT
    one-hot dst masks (plain DMA; no device gather).  The softmax max-shift
    is dropped (shift invariance; logits are O(10), safe in fp32 exp range);
    the denominator comes from a mask.T @ w matmul and is applied at PSUM
    evacuation.  Self-loops guarantee den > 0, so no epsilon term.
  - 6 SPMD stages: proj1 (as/ad), agg1(+W1 proj), proj2 (elu+W2|as2|ad2),
    agg2, proj3 (elu+W3|as3|ad3), agg3.  Host glue does layout only
    (permutation, row gathers by edge index, transpose, pad).
"""
import sys

sys.path.insert(0, "/opt/trn_rl_repo")

import heapq
import numpy as np
import ml_dtypes
from contextlib import ExitStack

import concourse.bass as bass
import concourse.bacc as bacc
import concourse.tile as tile
import concourse.mybir as mybir
from concourse.bass_utils import run_bass_kernel_spmd

F32 = mybir.dt.float32
F32R = mybir.dt.float32r
BF16 = mybir.dt.bfloat16
AL = mybir.AluOpType
AF = mybir.ActivationFunctionType
BF = ml_dtypes.bfloat16

N = 100000
NPAD = 100352            # 8 * 98 * 128
NCORE = 8
SHARD = NPAD // NCORE    # 12544
NBLK = SHARD // 128      # 98
NBLK_ALL = NPAD // 128   # 784
NEG = 0.2

_program_cache = {}
_last_stage_times = {}   # stage -> exec_time_ns (filled when trace=True)
TRACE = False
_shim_done = [False]


def _install_profile_shim():
    """The agent image's antenv lacks axon_hooks; recreate the tiny shim so
    run_bass_kernel_spmd(trace=True) can drive NTFF profiling via the axon
    plugin's C ABI, and stub the S3 artifact upload (no creds here)."""
    if _shim_done[0]:
        return
    import types
    mod = types.ModuleType("antenv.axon_hooks")
    holder = [None]
    mod.set_axon_ntff_profile_hook = lambda h: holder.__setitem__(0, h)
    mod.get_axon_ntff_profile_hook = lambda: holder[0]
    sys.modules["antenv.axon_hooks"] = mod
    from trn_agent_boot.trn_boot import _ntff_profile_via_ctypes
    holder[0] = _ntff_profile_via_ctypes('/opt/axon/libaxon_pjrt.so')
    import concourse.bass_utils as bu
    bu.upload_artifacts = lambda tmpdir: "local://" + str(tmpdir)
    _shim_done[0] = True


# --------------------------------------------------------------------------
# host-side preprocessing (indices / layout only)
# --------------------------------------------------------------------------

def _balance_slots(deg):
    """LPT bin-pack: assign nodes (by desc in-degree) to 784 blocks of 128
    slots so block edge counts are ~equal.  Returns slot_of_node[NPAD]."""
    order = np.argsort(-deg, kind="stable")
    heap = [(0, 0, b) for b in range(NBLK_ALL)]
    heapq.heapify(heap)
    slot_of_node = np.empty(NPAD, np.int64)
    fill = np.zeros(NBLK_ALL, np.int32)
    # process in runs of equal degree: round-robin via heap
    for n in order:
        s, c, b = heapq.heappop(heap)
        slot_of_node[n] = b * 128 + fill[b]
        fill[b] += 1
        c += 1
        if c < 128:
            heapq.heappush(heap, (s + int(deg[n]), c, b))
    return slot_of_node


def _plan(edge_index):
    src = np.concatenate([np.asarray(edge_index[0], np.int64),
                          np.arange(N, dtype=np.int64)])
    dst = np.concatenate([np.asarray(edge_index[1], np.int64),
                          np.arange(N, dtype=np.int64)])
    deg = np.bincount(dst, minlength=NPAD)  # padded "nodes" N..NPAD-1: deg 0
    slot_of_node = _balance_slots(deg)
    node_of_slot = np.empty(NPAD, np.int64)
    node_of_slot[slot_of_node] = np.arange(NPAD)

    dslot = slot_of_node[dst]
    blk = dslot // 128
    order = np.argsort(blk * 256 + (dslot % 128) // 64, kind="stable")
    src, dslot, blk = src[order], dslot[order], blk[order]
    bc = np.bincount(blk, minlength=NBLK_ALL)
    cmax = int(np.ceil(bc.max() / 128))
    starts = np.zeros(NBLK_ALL + 1, np.int64)
    np.cumsum(bc, out=starts[1:])
    L = cmax * 128

    # token slot for each edge: block-local position + block base
    within = np.arange(len(src)) - starts[blk]
    tok = blk * L + within                      # global padded token index
    T_all = NBLK_ALL * L
    tok_src = np.full(T_all, N, np.int64)       # N -> zero row in tables
    tok_dstl = np.full(T_all, 0, np.int64)
    tok_valid = np.zeros(T_all, bool)
    tok_src[tok] = src
    tok_dstl[tok] = dslot % 128
    tok_valid[tok] = True

    Tc = NBLK * L                               # tokens per core
    cores = []
    for c in range(NCORE):
        sl = slice(c * Tc, (c + 1) * Tc)
        cores.append(dict(src=tok_src[sl], dstl=tok_dstl[sl],
                          valid=tok_valid[sl]))
    return cores, cmax, slot_of_node, node_of_slot


def _chunkmaj(rows):
    """[T, C] row-major -> [128, T/128 * C] token-partition-major."""
    Tn, C = rows.shape
    ch = rows.reshape(Tn // 128, 128, C).transpose(1, 0, 2).reshape(128, -1)
    return np.ascontiguousarray(ch)


def _stream(core_plan, table, dtype):
    """Host row-gather by token src id -> [128, T/128 * C]."""
    Tn = core_plan["src"].shape[0]
    C = table.shape[1]
    rows = np.zeros((Tn, C), dtype)
    v = core_plan["valid"]
    rows[v] = table[core_plan["src"][v]].astype(dtype)
    return _chunkmaj(rows)


def _mask_stream(core_plan):
    """Host-built one-hot dst masks [128, T/128*128] bf16."""
    Tn = core_plan["dstl"].shape[0]
    rows = np.zeros((Tn, 128), BF)
    v = np.nonzero(core_plan["valid"])[0]
    rows[v, core_plan["dstl"][v]] = 1
    return _chunkmaj(rows)


def _we_stream(core_plan, src_tab, dst_tab, core_id, cmax):
    """[as(src) | ad(dst)] per token -> [128, T/128 * 2H] f32.
    src_tab/dst_tab are [NPAD(+1), H] node-indexed (row N.. = zeros)."""
    Tn = core_plan["src"].shape[0]
    H = src_tab.shape[1]
    rows = np.zeros((Tn, 2 * H), np.float32)
    v = core_plan["valid"]
    rows[v, :H] = src_tab[core_plan["src"][v]]
    L = cmax * 128
    blkl = np.arange(Tn) // L
    dst_slot = (core_id * NBLK + blkl) * 128 + core_plan["dstl"]
    rows[v, H:] = dst_tab[dst_slot[v]]          # dst_tab slot-indexed
    return _chunkmaj(rows)


# --------------------------------------------------------------------------
# stage program builders
# --------------------------------------------------------------------------

def _build_proj1():
    """asadT[8, SHARD] = ([ws1|wd1].T @ xT) with W stationary (f32r)."""
    GB = 4                                   # blocks per group (psum 4*128)
    ngrp = (NBLK + GB - 1) // GB
    nc = bacc.Bacc("TRN2", target_bir_lowering=False, debug=False,
                   num_devices=NCORE)
    xT = nc.dram_tensor("xT", [128, SHARD], F32, kind="ExternalInput").ap()
    W = nc.dram_tensor("W", [128, 8], F32, kind="ExternalInput").ap()
    out = nc.dram_tensor("out", [8, SHARD], F32, kind="ExternalOutput").ap()

    with tile.TileContext(nc) as tc, ExitStack() as ctx:
        cpool = ctx.enter_context(tc.tile_pool(name="c", bufs=1))
        pool = ctx.enter_context(tc.tile_pool(name="p", bufs=3))
        psum = ctx.enter_context(tc.tile_pool(name="ps", bufs=2, space="PSUM"))
        wt = cpool.tile([128, 8], F32)
        nc.sync.dma_start(wt[:], W[:])
        xTr = xT.rearrange("p (b n) -> p b n", n=128)
        for g in range(ngrp):
            nb = min(GB, NBLK - g * GB)
            xt = pool.tile([128, GB, 128], F32, tag="xt")
            nc.sync.dma_start(xt[:, 0:nb, :], xTr[:, g * GB:g * GB + nb, :])
            ps = psum.tile([8, GB * 128], F32, tag="ps")
            for j in range(nb):
                nc.tensor.matmul(ps[:, j * 128:(j + 1) * 128],
                                 wt[:], xt[:, j, :],
                                 start=True, stop=True)
            ot = pool.tile([8, GB * 128], F32, tag="ot")
            nc.vector.tensor_copy(ot[:, 0:nb * 128], ps[:, 0:nb * 128])
            nc.sync.dma_start(out[:, g * GB * 128:g * GB * 128 + nb * 128],
                              ot[:, 0:nb * 128])
    nc.compile()
    return nc


def _build_agg1(cmax):
    """agg1 + W1 proj: x2raw[dst, 512] = (sum alpha x_src) @ W1 + b1
    (no elu here; proj2 applies it)."""
    CM = cmax
    SB = 2                                    # blocks per super-iteration
    nc = bacc.Bacc("TRN2", target_bir_lowering=False, debug=False,
                   num_devices=NCORE)
    fe = nc.dram_tensor("fe", [128, NBLK * CM * 128], BF16,
                        kind="ExternalInput").ap()
    we = nc.dram_tensor("we", [128, NBLK * CM * 8], F32,
                        kind="ExternalInput").ap()
    mk = nc.dram_tensor("mk", [128, NBLK * CM * 128], BF16,
                        kind="ExternalInput").ap()
    W1b = nc.dram_tensor("W1b", [128, 512], BF16, kind="ExternalInput").ap()
    bt1 = nc.dram_tensor("bt1", [128, 512], BF16, kind="ExternalInput").ap()
    out = nc.dram_tensor("out", [SHARD, 512], BF16, kind="ExternalOutput").ap()

    with tile.TileContext(nc) as tc, ExitStack() as ctx:
        cpool = ctx.enter_context(tc.tile_pool(name="c", bufs=1))
        spool = ctx.enter_context(tc.tile_pool(name="s", bufs=3))
        wpool = ctx.enter_context(tc.tile_pool(name="w", bufs=2))
        epool = ctx.enter_context(tc.tile_pool(name="e", bufs=2))
        pagg = ctx.enter_context(tc.tile_pool(name="pa", bufs=2, space="PSUM"))
        pden = ctx.enter_context(tc.tile_pool(name="pd", bufs=2, space="PSUM"))
        po1 = ctx.enter_context(tc.tile_pool(name="po", bufs=2, space="PSUM"))

        w1t = cpool.tile([128, 512], BF16)
        nc.sync.dma_start(w1t[:], W1b[:])
        btt = cpool.tile([128, 512], BF16)
        nc.sync.dma_start(btt[:], bt1[:])

        nsb = NBLK // SB
        for sb in range(nsb):
            c0 = sb * SB * CM
            nch = SB * CM
            fet = spool.tile([128, nch, 128], BF16, tag="fet")
            nc.sync.dma_start(fet[:], fe[:, c0 * 128:(c0 + nch) * 128])
            wet = spool.tile([128, nch, 8], F32, tag="wet")
            nc.sync.dma_start(wet[:], we[:, c0 * 8:(c0 + nch) * 8])
            mkt = spool.tile([128, nch, 128], BF16, tag="mkt")
            nc.sync.dma_start(mkt[:], mk[:, c0 * 128:(c0 + nch) * 128])

            zt = wpool.tile([128, nch, 4], F32, tag="zt")
            nc.vector.tensor_add(zt[:], wet[:, :, 0:4], wet[:, :, 4:8])
            nc.vector.scalar_tensor_tensor(zt[:], zt[:], NEG, zt[:],
                                           AL.mult, AL.max)
            wf = wpool.tile([128, nch, 4], F32, tag="wf")
            nc.scalar.activation(wf[:], zt[:], AF.Exp)
            wb = wpool.tile([128, nch, 4], BF16, tag="wb")
            nc.vector.tensor_copy(wb[:], wf[:])

            wm = wpool.tile([128, nch, 4, 128], BF16, tag="wm")
            k = 0
            for ci in range(nch):
                for h in range(4):
                    eng = nc.vector if (k % 2 == 0) else nc.gpsimd
                    eng.tensor_scalar_mul(wm[:, ci, h, :], mkt[:, ci, :],
                                          wf[:, ci, h:h + 1])
                    k += 1

            for b in range(SB):
                aggT = pagg.tile([128, 512], F32, tag="aggT")
                den = pden.tile([128, 4], F32, tag="den")
                for ci in range(CM):
                    cc = b * CM + ci
                    nc.tensor.matmul(
                        aggT[:], fet[:, cc, :],
                        wm[:, cc, :, :].rearrange("p h d -> p (h d)"),
                        start=(ci == 0), stop=(ci == CM - 1))
                for ci in range(CM):
                    cc = b * CM + ci
                    nc.tensor.matmul(den[:], mkt[:, cc, :], wb[:, cc, :],
                                     start=(ci == 0), stop=(ci == CM - 1),
                                     skip_group_check=True)
                recip = epool.tile([128, 4], F32, tag="recip")
                nc.vector.reciprocal(recip[:], den[:])
                aggTs = epool.tile([128, 512], BF16, tag="aggTs")
                nc.vector.tensor_copy(aggTs[:, 0:256], aggT[:, 0:256])
                nc.scalar.activation(aggTs[:, 256:512], aggT[:, 256:512],
                                     AF.Copy)
                o1 = po1.tile([128, 512], F32, tag="o1")
                for h in range(4):
                    sl = slice(h * 128, (h + 1) * 128)
                    nc.tensor.matmul(o1[:, sl], aggTs[:, sl], w1t[:, sl],
                                     start=True, stop=True)
                xo = epool.tile([128, 512], BF16, tag="xo")
                for h in range(2):
                    sl = slice(h * 128, (h + 1) * 128)
                    nc.vector.scalar_tensor_tensor(
                        xo[:, sl], o1[:, sl], recip[:, h:h + 1], btt[:, sl],
                        AL.mult, AL.add)
                for h in range(2, 4):
                    sl = slice(h * 128, (h + 1) * 128)
                    nc.scalar.activation(xo[:, sl], o1[:, sl], AF.Copy,
                                         scale=recip[:, h:h + 1])
                nc.vector.tensor_add(xo[:, 256:512], xo[:, 256:512],
                                     btt[:, 256:512])
                blk = sb * SB + b
                nc.sync.dma_start(out[blk * 128:(blk + 1) * 128, :], xo[:])
    nc.compile()
    return nc


def _build_proj2():
    """h2sc = elu(x2raw).T-proj: in x2T [512, SHARD] bf16;
    out h2 [SHARD, 256] bf16 + sc [SHARD, 8] f32 (as2|ad2)."""
    nc = bacc.Bacc("TRN2", target_bir_lowering=False, debug=False,
                   num_devices=NCORE)
    xT = nc.dram_tensor("xT", [512, SHARD], BF16, kind="ExternalInput").ap()
    W = nc.dram_tensor("W", [512, 264], BF16, kind="ExternalInput").ap()
    oh = nc.dram_tensor("oh", [SHARD, 256], BF16, kind="ExternalOutput").ap()
    osc = nc.dram_tensor("osc", [SHARD, 8], F32, kind="ExternalOutput").ap()

    with tile.TileContext(nc) as tc, ExitStack() as ctx:
        cpool = ctx.enter_context(tc.tile_pool(name="c", bufs=1))
        pool = ctx.enter_context(tc.tile_pool(name="p", bufs=3))
        epool = ctx.enter_context(tc.tile_pool(name="e", bufs=2))
        psum = ctx.enter_context(tc.tile_pool(name="ps", bufs=2, space="PSUM"))
        wt = cpool.tile([128, 4, 264], BF16)
        for k in range(4):
            nc.sync.dma_start(wt[:, k, :], W[128 * k:128 * (k + 1), :])
        xTr = xT.rearrange("(k p) n -> p k n", p=128)
        GB = 8                                   # blocks of sc staged per DMA
        for bi in range(NBLK):
            xt = pool.tile([128, 4, 128], BF16, tag="xt")
            nc.sync.dma_start(xt[:], xTr[:, :, 128 * bi:(bi + 1) * 128])
            # elu in place: xe = max(exp(min(x,0))-1, x)
            mt = pool.tile([128, 4, 128], BF16, tag="mt")
            nc.gpsimd.tensor_scalar_min(mt[:], xt[:], 0.0)
            nc.scalar.activation(mt[:], mt[:], AF.Exp)
            nc.vector.scalar_tensor_tensor(xt[:], mt[:], 1.0, xt[:],
                                           AL.subtract, AL.max)
            ps = psum.tile([128, 264], F32, tag="ps")
            for k in range(4):
                nc.tensor.matmul(ps[:], xt[:, k, :], wt[:, k, :],
                                 start=(k == 0), stop=(k == 3))
            ht = epool.tile([128, 256], BF16, tag="ht")
            nc.vector.tensor_copy(ht[:, 0:128], ps[:, 0:128])
            nc.scalar.activation(ht[:, 128:256], ps[:, 128:256], AF.Copy)
            nc.sync.dma_start(oh[bi * 128:(bi + 1) * 128, :], ht[:])
            g, j = bi // GB, bi % GB
            if j == 0:
                sct = epool.tile([128, GB, 8], F32, tag="sct")
            nc.vector.tensor_copy(sct[:, j, :], ps[:, 256:264])
            if j == GB - 1 or bi == NBLK - 1:
                nb = j + 1
                oscr = osc.rearrange("(b p) c -> p b c", p=128)
                nc.sync.dma_start(oscr[:, g * GB:g * GB + nb, :],
                                  sct[:, 0:nb, :])
    nc.compile()
    return nc


def _build_agg2(cmax):
    """agg2: x3raw[dst, 256] = (sum alpha h2_src) + b2 (no elu; proj3)."""
    CM = cmax
    SB = 2
    nc = bacc.Bacc("TRN2", target_bir_lowering=False, debug=False,
                   num_devices=NCORE)
    fe = nc.dram_tensor("fe", [128, NBLK * CM * 256], BF16,
                        kind="ExternalInput").ap()
    we = nc.dram_tensor("we", [128, NBLK * CM * 8], F32,
                        kind="ExternalInput").ap()
    mk = nc.dram_tensor("mk", [128, NBLK * CM * 128], BF16,
                        kind="ExternalInput").ap()
    bt2 = nc.dram_tensor("bt2", [128, 256], BF16, kind="ExternalInput").ap()
    out = nc.dram_tensor("out", [SHARD, 256], BF16, kind="ExternalOutput").ap()

    with tile.TileContext(nc) as tc, ExitStack() as ctx:
        cpool = ctx.enter_context(tc.tile_pool(name="c", bufs=1))
        spool = ctx.enter_context(tc.tile_pool(name="s", bufs=3))
        wpool = ctx.enter_context(tc.tile_pool(name="w", bufs=2))
        epool = ctx.enter_context(tc.tile_pool(name="e", bufs=2))
        pagg = ctx.enter_context(tc.tile_pool(name="pa", bufs=3, space="PSUM"))

        btt = cpool.tile([128, 256], BF16)
        nc.sync.dma_start(btt[:], bt2[:])

        nsb = NBLK // SB
        for sb in range(nsb):
            c0 = sb * SB * CM
            nch = SB * CM
            # fs laid out [128, nch, 264]: cols 0:256 scaled features,
            # cols 256:260 = w (bf16) so one rhs serves agg+den.
            fet = spool.tile([128, nch, 256], BF16, tag="fet")
            nc.sync.dma_start(fet[:], fe[:, c0 * 256:(c0 + nch) * 256])
            wet = spool.tile([128, nch, 8], F32, tag="wet")
            nc.sync.dma_start(wet[:], we[:, c0 * 8:(c0 + nch) * 8])
            mkt = spool.tile([128, nch, 128], BF16, tag="mkt")
            nc.sync.dma_start(mkt[:], mk[:, c0 * 128:(c0 + nch) * 128])

            zt = wpool.tile([128, nch, 4], F32, tag="zt")
            nc.vector.tensor_add(zt[:], wet[:, :, 0:4], wet[:, :, 4:8])
            nc.vector.scalar_tensor_tensor(zt[:], zt[:], NEG, zt[:],
                                           AL.mult, AL.max)
            wf = wpool.tile([128, nch, 4], F32, tag="wf")
            nc.scalar.activation(wf[:], zt[:], AF.Exp)

            fs = wpool.tile([128, nch, 264], BF16, tag="fs")
            nc.vector.tensor_copy(fs[:, :, 256:260], wf[:])
            k = 0
            for ci in range(nch):
                for h in range(4):
                    eng = nc.vector if (k % 2 == 0) else nc.gpsimd
                    eng.tensor_scalar_mul(fs[:, ci, h * 64:(h + 1) * 64],
                                          fet[:, ci, h * 64:(h + 1) * 64],
                                          wf[:, ci, h:h + 1])
                    k += 1

            for b in range(SB):
                agg = pagg.tile([128, 260], F32, tag="agg")
                for ci in range(CM):
                    cc = b * CM + ci
                    nc.tensor.matmul(agg[:], mkt[:, cc, :],
                                     fs[:, cc, 0:260],
                                     start=(ci == 0), stop=(ci == CM - 1))
                recip = epool.tile([128, 4], F32, tag="recip")
                nc.vector.reciprocal(recip[:], agg[:, 256:260])
                xo = epool.tile([128, 256], BF16, tag="xo")
                for h in range(2):
                    sl = slice(h * 64, (h + 1) * 64)
                    nc.vector.scalar_tensor_tensor(
                        xo[:, sl], agg[:, sl], recip[:, h:h + 1], btt[:, sl],
                        AL.mult, AL.add)
                for h in range(2, 4):
                    sl = slice(h * 64, (h + 1) * 64)
                    nc.scalar.activation(xo[:, sl], agg[:, sl], AF.Copy,
                                         scale=recip[:, h:h + 1])
                nc.vector.tensor_add(xo[:, 128:256], xo[:, 128:256],
                                     btt[:, 128:256])
                blk = sb * SB + b
                nc.sync.dma_start(out[blk * 128:(blk + 1) * 128, :], xo[:])
    nc.compile()
    return nc


def _build_proj3():
    """h3scT = [W3|ws3|wd3].T @ elu(x3raw).T: in x3T [256, SHARD] bf16;
    out [4, SHARD] f32 (h3 2 | as3 | ad3), W stationary."""
    GB = 4
    ngrp = (NBLK + GB - 1) // GB
    nc = bacc.Bacc("TRN2", target_bir_lowering=False, debug=False,
                   num_devices=NCORE)
    xT = nc.dram_tensor("xT", [256, SHARD], BF16, kind="ExternalInput").ap()
    W = nc.dram_tensor("W", [256, 4], BF16, kind="ExternalInput").ap()
    out = nc.dram_tensor("out", [4, SHARD], F32, kind="ExternalOutput").ap()

    with tile.TileContext(nc) as tc, ExitStack() as ctx:
        cpool = ctx.enter_context(tc.tile_pool(name="c", bufs=1))
        pool = ctx.enter_context(tc.tile_pool(name="p", bufs=3))
        psum = ctx.enter_context(tc.tile_pool(name="ps", bufs=2, space="PSUM"))
        wt = cpool.tile([128, 2, 4], BF16)
        for k in range(2):
            nc.sync.dma_start(wt[:, k, :], W[128 * k:128 * (k + 1), :])
        xTr = xT.rearrange("(k p) n -> p k n", p=128)
        for g in range(ngrp):
            nb = min(GB, NBLK - g * GB)
            xt = pool.tile([128, 2, GB, 128], BF16, tag="xt")
            nc.sync.dma_start(xt[:, :, 0:nb, :],
                              xTr[:, :, g * GB * 128:(g * GB + nb) * 128]
                              .rearrange("p k (b n) -> p k b n", n=128))
            mt = pool.tile([128, 2, GB, 128], BF16, tag="mt")
            nc.gpsimd.tensor_scalar_min(mt[:, :, 0:nb, :], xt[:, :, 0:nb, :],
                                        0.0)
            nc.scalar.activation(mt[:, :, 0:nb, :], mt[:, :, 0:nb, :], AF.Exp)
            nc.vector.scalar_tensor_tensor(xt[:, :, 0:nb, :], mt[:, :, 0:nb, :],
                                           1.0, xt[:, :, 0:nb, :],
                                           AL.subtract, AL.max)
            ps = psum.tile([4, GB * 128], F32, tag="ps")
            for j in range(nb):
                for k in range(2):
                    nc.tensor.matmul(ps[:, j * 128:(j + 1) * 128],
                                     wt[:, k, :], xt[:, k, j, :],
                                     start=(k == 0), stop=(k == 1))
            ot = pool.tile([4, GB * 128], F32, tag="ot")
            nc.vector.tensor_copy(ot[:, 0:nb * 128], ps[:, 0:nb * 128])
            nc.sync.dma_start(out[:, g * GB * 128:g * GB * 128 + nb * 128],
                              ot[:, 0:nb * 128])
    nc.compile()
    return nc


def _build_agg3(cmax):
    """agg3: out[dst, 2] = (sum alpha h3_src) + b3, heads=1."""
    CM = cmax
    SB = 4
    nc = bacc.Bacc("TRN2", target_bir_lowering=False, debug=False,
                   num_devices=NCORE)
    fe = nc.dram_tensor("fe", [128, NBLK * CM * 2], BF16,
                        kind="ExternalInput").ap()
    we = nc.dram_tensor("we", [128, NBLK * CM * 2], F32,
                        kind="ExternalInput").ap()
    mk = nc.dram_tensor("mk", [128, NBLK * CM * 128], BF16,
                        kind="ExternalInput").ap()
    bt3 = nc.dram_tensor("bt3", [128, 2], F32, kind="ExternalInput").ap()
    out = nc.dram_tensor("out", [SHARD, 2], F32, kind="ExternalOutput").ap()

    with tile.TileContext(nc) as tc, ExitStack() as ctx:
        cpool = ctx.enter_context(tc.tile_pool(name="c", bufs=1))
        spool = ctx.enter_context(tc.tile_pool(name="s", bufs=3))
        wpool = ctx.enter_context(tc.tile_pool(name="w", bufs=2))
        epool = ctx.enter_context(tc.tile_pool(name="e", bufs=2))
        pagg = ctx.enter_context(tc.tile_pool(name="pa", bufs=3, space="PSUM"))

        btt = cpool.tile([128, 2], F32)
        nc.sync.dma_start(btt[:], bt3[:])

        nsb = NBLK // SB + (1 if NBLK % SB else 0)
        for sb in range(nsb):
            b0 = sb * SB
            nb = min(SB, NBLK - b0)
            c0 = b0 * CM
            nch = nb * CM
            fet = spool.tile([128, SB * CM, 2], BF16, tag="fet")
            nc.sync.dma_start(fet[:, 0:nch, :], fe[:, c0 * 2:(c0 + nch) * 2])
            wet = spool.tile([128, SB * CM, 2], F32, tag="wet")
            nc.sync.dma_start(wet[:, 0:nch, :], we[:, c0 * 2:(c0 + nch) * 2])
            mkt = spool.tile([128, SB * CM, 128], BF16, tag="mkt")
            nc.sync.dma_start(mkt[:, 0:nch, :],
                              mk[:, c0 * 128:(c0 + nch) * 128])

            zt = wpool.tile([128, SB * CM, 1], F32, tag="zt")
            nc.vector.tensor_add(zt[:, 0:nch, :], wet[:, 0:nch, 0:1],
                                 wet[:, 0:nch, 1:2])
            nc.vector.scalar_tensor_tensor(zt[:, 0:nch, :], zt[:, 0:nch, :],
                                           NEG, zt[:, 0:nch, :],
                                           AL.mult, AL.max)
            wf = wpool.tile([128, SB * CM, 1], F32, tag="wf")
            nc.scalar.activation(wf[:, 0:nch, :], zt[:, 0:nch, :], AF.Exp)
            rhs = wpool.tile([128, SB * CM, 4], BF16, tag="rhs")
            nc.vector.tensor_tensor(
                rhs[:, 0:nch, 0:2], fet[:, 0:nch, :],
                wf[:, 0:nch, :].broadcast_to([128, nch, 2]), AL.mult)
            nc.vector.tensor_copy(rhs[:, 0:nch, 2:3], wf[:, 0:nch, :])

            xs = epool.tile([128, SB, 2], F32, tag="xs")
            for b in range(nb):
                agg = pagg.tile([128, 3], F32, tag="agg")
                for ci in range(CM):
                    cc = b * CM + ci
                    nc.tensor.matmul(agg[:], mkt[:, cc, :], rhs[:, cc, 0:3],
                                     start=(ci == 0), stop=(ci == CM - 1))
                recip = epool.tile([128, 1], F32, tag="recip")
                nc.vector.reciprocal(recip[:], agg[:, 2:3])
                nc.vector.scalar_tensor_tensor(
                    xs[:, b, :], agg[:, 0:2], recip[:], btt[:],
                    AL.mult, AL.add)
            outr = out.rearrange("(b p) c -> p b c", p=128)
            nc.sync.dma_start(outr[:, b0:b0 + nb, :], xs[:, 0:nb, :])
    nc.compile()
    return nc


# --------------------------------------------------------------------------
# orchestration
# --------------------------------------------------------------------------

def _get_program(key, builder):
    if key not in _program_cache:
        _program_cache[key] = builder()
    return _program_cache[key]


def _run(stage, nc, in_maps):
    if TRACE:
        _install_profile_shim()
    res = run_bass_kernel_spmd(nc, in_maps, core_ids=list(range(NCORE)),
                               trace=TRACE, trace_cores=list(range(NCORE)),
                               stitch_traces=False)
    if res.exec_time_ns is not None:
        _last_stage_times[stage] = res.exec_time_ns
    return res


def _fold_ws(W, a, heads, out_c):
    Wr = W.reshape(W.shape[0], heads, out_c)
    return np.einsum('fhc,hc->fh', Wr.astype(np.float64),
                     a.astype(np.float64)).astype(np.float32)


def _bcast(b, outc, dtype):
    return np.ascontiguousarray(
        np.broadcast_to(np.asarray(b, np.float32).reshape(1, outc),
                        (128, outc)).astype(dtype))


def _pad_tab(tab):
    """[N, C] node table -> [NPAD+1, C] with zero pad rows (row N..: zeros)."""
    out = np.zeros((NPAD + 1, tab.shape[1]), tab.dtype)
    out[:tab.shape[0]] = tab
    return out


def kernel(x, edge_index, W1, a_src1, a_dst1, b1, W2, a_src2, a_dst2, b2,
           W3, a_src3, a_dst3, b3):
    x = np.asarray(x, np.float32)
    W1 = np.asarray(W1, np.float32); W2 = np.asarray(W2, np.float32)
    W3 = np.asarray(W3, np.float32)

    cores, cmax, slot_of_node, node_of_slot = _plan(np.asarray(edge_index))

    ws1 = _fold_ws(W1, np.asarray(a_src1, np.float32), 4, 128)
    wd1 = _fold_ws(W1, np.asarray(a_dst1, np.float32), 4, 128)
    ws2 = _fold_ws(W2, np.asarray(a_src2, np.float32), 4, 64)
    wd2 = _fold_ws(W2, np.asarray(a_dst2, np.float32), 4, 64)
    ws3 = _fold_ws(W3, np.asarray(a_src3, np.float32), 1, 2)
    wd3 = _fold_ws(W3, np.asarray(a_dst3, np.float32), 1, 2)

    # node features in slot order (xs[slot] = x[node_of_slot[slot]])
    xs = np.zeros((NPAD, 128), np.float32)
    real = node_of_slot < N
    xs[real] = x[node_of_slot[real]]

    masks = [_mask_stream(c) for c in cores]

    # stage 1: as1/ad1 for all slots (shard-local rows)
    s1 = _get_program("proj1", _build_proj1)
    wsd1 = np.concatenate([ws1, wd1], axis=1)
    r1 = _run("proj1", s1, [
        {"xT": np.ascontiguousarray(xs[c * SHARD:(c + 1) * SHARD].T),
         "W": wsd1} for c in range(NCORE)])
    asad1 = np.concatenate([r1.results[c]["out"].T for c in range(NCORE)])
    # as1 by node id: asad1 is slot-indexed
    as1_node = np.zeros((NPAD + 1, 4), np.float32)
    as1_node[node_of_slot] = asad1[:, 0:4]
    ad1_slot = asad1[:, 4:8]

    # stage 2: agg1 + W1 proj -> x2raw (slot-major [NPAD, 512] bf16)
    xtab = np.zeros((NPAD + 1, 128), BF)       # node-indexed feature table
    xtab[:N] = x.astype(BF)
    s2 = _get_program(("agg1", cmax), lambda: _build_agg1(cmax))
    in2 = [{"fe": _stream(cores[c], xtab, BF),
            "we": _we_stream(cores[c], as1_node, ad1_slot, c, cmax),
            "mk": masks[c],
            "W1b": W1.astype(BF),
            "bt1": _bcast(b1, 512, BF)} for c in range(NCORE)]
    r2 = _run("agg1", s2, in2)
    x2 = np.concatenate([r2.results[c]["out"] for c in range(NCORE)])

    # stage 3: proj2 (elu + W2|ws2|wd2)
    s3 = _get_program("proj2", _build_proj2)
    W2s = np.concatenate([W2, ws2, wd2], axis=1).astype(BF)
    r3 = _run("proj2", s3, [
        {"xT": np.ascontiguousarray(x2[c * SHARD:(c + 1) * SHARD].T),
         "W": W2s} for c in range(NCORE)])
    h2 = np.concatenate([r3.results[c]["oh"] for c in range(NCORE)])
    sc2 = np.concatenate([r3.results[c]["osc"] for c in range(NCORE)])
    as2_node = np.zeros((NPAD + 1, 4), np.float32)
    as2_node[node_of_slot] = sc2[:, 0:4]
    ad2_slot = sc2[:, 4:8]

    # stage 4: agg2 -> x3raw
    h2tab = np.zeros((NPAD + 1, 256), BF)
    h2tab[node_of_slot] = h2
    s4 = _get_program(("agg2", cmax), lambda: _build_agg2(cmax))
    in4 = [{"fe": _stream(cores[c], h2tab, BF),
            "we": _we_stream(cores[c], as2_node, ad2_slot, c, cmax),
            "mk": masks[c],
            "bt2": _bcast(b2, 256, BF)} for c in range(NCORE)]
    r4 = _run("agg2", s4, in4)
    x3 = np.concatenate([r4.results[c]["out"] for c in range(NCORE)])

    # stage 5: proj3 (elu + W3|ws3|wd3), W stationary, transposed out
    s5 = _get_program("proj3", _build_proj3)
    W3s = np.concatenate([W3, ws3, wd3], axis=1).astype(BF)
    r5 = _run("proj3", s5, [
        {"xT": np.ascontiguousarray(x3[c * SHARD:(c + 1) * SHARD].T),
         "W": W3s} for c in range(NCORE)])
    h3sc = np.concatenate([r5.results[c]["out"].T for c in range(NCORE)])
    h3_slot, as3_slot, ad3_slot = (h3sc[:, 0:2], h3sc[:, 2:3], h3sc[:, 3:4])
    h3tab = np.zeros((NPAD + 1, 2), BF)
    h3tab[node_of_slot] = h3_slot
    as3_node = np.zeros((NPAD + 1, 1), np.float32)
    as3_node[node_of_slot] = as3_slot

    # stage 6: agg3 -> out
    s6 = _get_program(("agg3", cmax), lambda: _build_agg3(cmax))
    in6 = [{"fe": _stream(cores[c], h3tab, BF),
            "we": _we_stream(cores[c], as3_node, ad3_slot, c, cmax),
            "mk": masks[c],
            "bt3": _bcast(b3, 2, np.float32)} for c in range(NCORE)]
    r6 = _run("agg3", s6, in6)
    outp = np.concatenate([r6.results[c]["out"] for c in range(NCORE)])
    return np.ascontiguousarray(outp[slot_of_node[:N]]).astype(np.float32)


# revision 10
# speedup vs baseline: 3.5188x; 3.5188x over previous
"""3-layer GAT (PyG GATConv-style) on 8 Trainium2 NeuronCores — v2.

Strategy (dst-node sharding, all fp math on device):
  - Nodes are permuted into 100352 slots = 8 shards x 98 blocks x 128 via an
    LPT bin-pack on in-degree so every block has ~equal incoming-edge count
    (cmax = ceil(max_block_edges/128) drops 6 -> 5).
  - Edges (incl. self-loops) are partitioned by dst block; per (core, block)
    the edge list is padded to cmax 128-edge chunks; all 8 cores run one SPMD
    program per stage.
  - Segment softmax + weighted segment-sum run on TensorE via HOST-BUILT
    one-hot dst masks (plain DMA; no device gather).  The softmax max-shift
    is dropped (shift invariance; logits are O(10), safe in fp32 exp range);
    the denominator comes from a mask.T @ w matmul and is applied at PSUM
    evacuation.  Self-loops guarantee den > 0, so no epsilon term.
  - 6 SPMD stages: proj1 (as/ad), agg1(+W1 proj), proj2 (elu+W2|as2|ad2),
    agg2, proj3 (elu+W3|as3|ad3), agg3.  Host glue does layout only
    (permutation, row gathers by edge index, transpose, pad).
"""
import sys

sys.path.insert(0, "/opt/trn_rl_repo")

import heapq
import numpy as np
import ml_dtypes
from contextlib import ExitStack

import concourse.bass as bass
import concourse.bacc as bacc
import concourse.tile as tile
import concourse.mybir as mybir
from concourse.bass_utils import run_bass_kernel_spmd

F32 = mybir.dt.float32
F32R = mybir.dt.float32r
BF16 = mybir.dt.bfloat16
AL = mybir.AluOpType
AF = mybir.ActivationFunctionType
BF = ml_dtypes.bfloat16

N = 100000
NPAD = 100352            # 8 * 98 * 128
NCORE = 8
SHARD = NPAD // NCORE    # 12544
NBLK = SHARD // 128      # 98
NBLK_ALL = NPAD // 128   # 784
NEG = 0.2

_program_cache = {}
_last_stage_times = {}   # stage -> exec_time_ns (filled when trace=True)
TRACE = False
_shim_done = [False]


def _install_profile_shim():
    """The agent image's antenv lacks axon_hooks; recreate the tiny shim so
    run_bass_kernel_spmd(trace=True) can drive NTFF profiling via the axon
    plugin's C ABI, and stub the S3 artifact upload (no creds here)."""
    if _shim_done[0]:
        return
    import types
    mod = types.ModuleType("antenv.axon_hooks")
    holder = [None]
    mod.set_axon_ntff_profile_hook = lambda h: holder.__setitem__(0, h)
    mod.get_axon_ntff_profile_hook = lambda: holder[0]
    sys.modules["antenv.axon_hooks"] = mod
    from trn_agent_boot.trn_boot import _ntff_profile_via_ctypes
    holder[0] = _ntff_profile_via_ctypes('/opt/axon/libaxon_pjrt.so')
    import concourse.bass_utils as bu
    bu.upload_artifacts = lambda tmpdir: "local://" + str(tmpdir)
    _shim_done[0] = True


# --------------------------------------------------------------------------
# host-side preprocessing (indices / layout only)
# --------------------------------------------------------------------------

def _balance_slots(deg):
    """LPT bin-pack: assign nodes (by desc in-degree) to 784 blocks of 128
    slots so block edge counts are ~equal.  Returns slot_of_node[NPAD]."""
    order = np.argsort(-deg, kind="stable")
    heap = [(0, 0, b) for b in range(NBLK_ALL)]
    heapq.heapify(heap)
    slot_of_node = np.empty(NPAD, np.int64)
    fill = np.zeros(NBLK_ALL, np.int32)
    # process in runs of equal degree: round-robin via heap
    for n in order:
        s, c, b = heapq.heappop(heap)
        slot_of_node[n] = b * 128 + fill[b]
        fill[b] += 1
        c += 1
        if c < 128:
            heapq.heappush(heap, (s + int(deg[n]), c, b))
    return slot_of_node


def _plan(edge_index):
    src = np.concatenate([np.asarray(edge_index[0], np.int64),
                          np.arange(N, dtype=np.int64)])
    dst = np.concatenate([np.asarray(edge_index[1], np.int64),
                          np.arange(N, dtype=np.int64)])
    deg = np.bincount(dst, minlength=NPAD)  # padded "nodes" N..NPAD-1: deg 0
    slot_of_node = _balance_slots(deg)
    node_of_slot = np.empty(NPAD, np.int64)
    node_of_slot[slot_of_node] = np.arange(NPAD)

    dslot = slot_of_node[dst]
    blk = dslot // 128
    order = np.argsort(blk * 256 + (dslot % 128) // 64, kind="stable")
    src, dslot, blk = src[order], dslot[order], blk[order]
    bc = np.bincount(blk, minlength=NBLK_ALL)
    cmax = int(np.ceil(bc.max() / 128))
    starts = np.zeros(NBLK_ALL + 1, np.int64)
    np.cumsum(bc, out=starts[1:])
    L = cmax * 128

    # token slot for each edge: block-local position + block base
    within = np.arange(len(src)) - starts[blk]
    tok = blk * L + within                      # global padded token index
    T_all = NBLK_ALL * L
    tok_src = np.full(T_all, N, np.int64)       # N -> zero row in tables
    tok_dstl = np.full(T_all, 0, np.int64)
    tok_valid = np.zeros(T_all, bool)
    tok_src[tok] = src
    tok_dstl[tok] = dslot % 128
    tok_valid[tok] = True

    Tc = NBLK * L                               # tokens per core
    cores = []
    for c in range(NCORE):
        sl = slice(c * Tc, (c + 1) * Tc)
        cores.append(dict(src=tok_src[sl], dstl=tok_dstl[sl],
                          valid=tok_valid[sl]))
    return cores, cmax, slot_of_node, node_of_slot


def _chunkmaj(rows):
    """[T, C] row-major -> [128, T/128 * C] token-partition-major."""
    Tn, C = rows.shape
    ch = rows.reshape(Tn // 128, 128, C).transpose(1, 0, 2).reshape(128, -1)
    return np.ascontiguousarray(ch)


def _stream(core_plan, table, dtype):
    """Host row-gather by token src id -> [128, T/128 * C]."""
    Tn = core_plan["src"].shape[0]
    C = table.shape[1]
    rows = np.zeros((Tn, C), dtype)
    v = core_plan["valid"]
    rows[v] = table[core_plan["src"][v]].astype(dtype)
    return _chunkmaj(rows)


def _mask_stream(core_plan):
    """Host-built one-hot dst masks [128, T/128*128] bf16."""
    Tn = core_plan["dstl"].shape[0]
    rows = np.zeros((Tn, 128), BF)
    v = np.nonzero(core_plan["valid"])[0]
    rows[v, core_plan["dstl"][v]] = 1
    return _chunkmaj(rows)


def _we_stream(core_plan, src_tab, dst_tab, core_id, cmax):
    """[as(src) | ad(dst)] per token -> [128, T/128 * 2H] f32.
    src_tab/dst_tab are [NPAD(+1), H] node-indexed (row N.. = zeros)."""
    Tn = core_plan["src"].shape[0]
    H = src_tab.shape[1]
    rows = np.zeros((Tn, 2 * H), np.float32)
    v = core_plan["valid"]
    rows[v, :H] = src_tab[core_plan["src"][v]]
    L = cmax * 128
    blkl = np.arange(Tn) // L
    dst_slot = (core_id * NBLK + blkl) * 128 + core_plan["dstl"]
    rows[v, H:] = dst_tab[dst_slot[v]]          # dst_tab slot-indexed
    return _chunkmaj(rows)


# --------------------------------------------------------------------------
# stage program builders
# --------------------------------------------------------------------------

def _build_proj1():
    """asadT[8, SHARD] = ([ws1|wd1].T @ xT) with W stationary (f32r)."""
    GB = 4                                   # blocks per group (psum 4*128)
    ngrp = (NBLK + GB - 1) // GB
    nc = bacc.Bacc("TRN2", target_bir_lowering=False, debug=False,
                   num_devices=NCORE)
    xT = nc.dram_tensor("xT", [128, SHARD], F32, kind="ExternalInput").ap()
    W = nc.dram_tensor("W", [128, 8], F32, kind="ExternalInput").ap()
    out = nc.dram_tensor("out", [8, SHARD], F32, kind="ExternalOutput").ap()

    with tile.TileContext(nc) as tc, ExitStack() as ctx:
        cpool = ctx.enter_context(tc.tile_pool(name="c", bufs=1))
        pool = ctx.enter_context(tc.tile_pool(name="p", bufs=3))
        psum = ctx.enter_context(tc.tile_pool(name="ps", bufs=2, space="PSUM"))
        wt = cpool.tile([128, 8], F32)
        nc.sync.dma_start(wt[:], W[:])
        xTr = xT.rearrange("p (b n) -> p b n", n=128)
        for g in range(ngrp):
            nb = min(GB, NBLK - g * GB)
            xt = pool.tile([128, GB, 128], F32, tag="xt")
            nc.sync.dma_start(xt[:, 0:nb, :], xTr[:, g * GB:g * GB + nb, :])
            ps = psum.tile([8, GB * 128], F32, tag="ps")
            for j in range(nb):
                nc.tensor.matmul(ps[:, j * 128:(j + 1) * 128],
                                 wt[:], xt[:, j, :],
                                 start=True, stop=True)
            ot = pool.tile([8, GB * 128], F32, tag="ot")
            nc.vector.tensor_copy(ot[:, 0:nb * 128], ps[:, 0:nb * 128])
            nc.sync.dma_start(out[:, g * GB * 128:g * GB * 128 + nb * 128],
                              ot[:, 0:nb * 128])
    nc.compile()
    return nc


def _build_agg1(cmax):
    """agg1 + W1 proj: x2raw[dst, 512] = (sum alpha x_src) @ W1 + b1
    (no elu here; proj2 applies it).  Software-pipelined: heavy PSUM-side
    work of block i is emitted after the stream work of block i+1."""
    CM = cmax
    WEB = 4                                   # blocks per we-DMA batch
    NACT = max(1, CM // 4)                    # wm chunks built on ScalarE
    NDVE = CM - NACT                          # wm chunks built on VectorE
    nc = bacc.Bacc("TRN2", target_bir_lowering=False, debug=False,
                   num_devices=NCORE)
    fe = nc.dram_tensor("fe", [128, NBLK * CM * 128], BF16,
                        kind="ExternalInput").ap()
    we = nc.dram_tensor("we", [128, NBLK * CM * 8], F32,
                        kind="ExternalInput").ap()
    mk = nc.dram_tensor("mk", [128, NBLK * CM * 128], BF16,
                        kind="ExternalInput").ap()
    W1b = nc.dram_tensor("W1b", [128, 512], BF16, kind="ExternalInput").ap()
    bt1 = nc.dram_tensor("bt1", [128, 512], BF16, kind="ExternalInput").ap()
    out = nc.dram_tensor("out", [SHARD, 512], BF16, kind="ExternalOutput").ap()

    with tile.TileContext(nc) as tc, ExitStack() as ctx:
        cpool = ctx.enter_context(tc.tile_pool(name="c", bufs=1))
        spool = ctx.enter_context(tc.tile_pool(name="s", bufs=3))
        vpool = ctx.enter_context(tc.tile_pool(name="v", bufs=2))
        wpool = ctx.enter_context(tc.tile_pool(name="w", bufs=3))
        epool = ctx.enter_context(tc.tile_pool(name="e", bufs=2))
        pagg = ctx.enter_context(tc.tile_pool(name="pa", bufs=2, space="PSUM"))
        pden = ctx.enter_context(tc.tile_pool(name="pd", bufs=2, space="PSUM"))
        po1 = ctx.enter_context(tc.tile_pool(name="po", bufs=2, space="PSUM"))

        w1t = cpool.tile([128, 512], BF16)
        nc.sync.dma_start(w1t[:], W1b[:])
        btt = cpool.tile([128, 512], BF16)
        nc.sync.dma_start(btt[:], bt1[:])

        state = {}

        def front(bi):
            c0 = bi * CM
            if bi % WEB == 0:
                nw = min(WEB, NBLK - bi) * CM
                wet = vpool.tile([128, WEB * CM, 8], F32, tag="wet")
                nc.sync.dma_start(wet[:, 0:nw, :], we[:, c0 * 8:(c0 + nw) * 8])
                zt = vpool.tile([128, WEB * CM, 4], F32, tag="zt")
                nc.vector.tensor_add(zt[:, 0:nw, :], wet[:, 0:nw, 0:4],
                                     wet[:, 0:nw, 4:8])
                nc.vector.scalar_tensor_tensor(zt[:, 0:nw, :], zt[:, 0:nw, :],
                                               NEG, zt[:, 0:nw, :],
                                               AL.mult, AL.max)
                wf = vpool.tile([128, WEB * CM, 4], F32, tag="wf")
                nc.scalar.activation(wf[:, 0:nw, :], zt[:, 0:nw, :], AF.Exp)
                wb = vpool.tile([128, WEB * CM, 4], BF16, tag="wb")
                nc.vector.tensor_copy(wb[:, 0:nw, :], wf[:, 0:nw, :])
                state["wf"], state["wb"] = wf, wb
            wf, wb = state["wf"], state["wb"]
            q = (bi % WEB) * CM
            fet = spool.tile([128, CM, 128], BF16, tag="fet")
            nc.sync.dma_start(fet[:], fe[:, c0 * 128:(c0 + CM) * 128])
            mkt = spool.tile([128, CM, 128], BF16, tag="mkt")
            nc.sync.dma_start(mkt[:], mk[:, c0 * 128:(c0 + CM) * 128])

            wm = wpool.tile([128, CM, 4, 128], BF16, tag="wm")
            nc.vector.tensor_tensor(
                wm[:, 0:NDVE, :, :],
                mkt[:, 0:NDVE, :].unsqueeze(2).broadcast_to(
                    [128, NDVE, 4, 128]),
                wf[:, q:q + NDVE, :].unsqueeze(3).broadcast_to(
                    [128, NDVE, 4, 128]),
                AL.mult)
            for ci in range(NDVE, CM):
                for h in range(4):
                    nc.scalar.activation(wm[:, ci, h, :], mkt[:, ci, :],
                                         AF.Copy, scale=wf[:, q + ci, h:h + 1])

            aggT = pagg.tile([128, 512], F32, tag="aggT")
            den = pden.tile([128, 4], F32, tag="den")
            for ci in range(CM):
                nc.tensor.matmul(
                    aggT[:], fet[:, ci, :],
                    wm[:, ci, :, :].rearrange("p h d -> p (h d)"),
                    start=(ci == 0), stop=(ci == CM - 1))
            for ci in range(CM):
                nc.tensor.matmul(den[:], mkt[:, ci, :], wb[:, q + ci, :],
                                 start=(ci == 0), stop=(ci == CM - 1),
                                 skip_group_check=True)
            return aggT, den

        def back(bi, aggT, den):
            recip = epool.tile([128, 4], F32, tag="recip")
            nc.vector.reciprocal(recip[:], den[:])
            aggTs = epool.tile([128, 512], BF16, tag="aggTs")
            nc.vector.tensor_copy(aggTs[:, 0:256], aggT[:, 0:256])
            nc.scalar.activation(aggTs[:, 256:512], aggT[:, 256:512], AF.Copy)
            o1 = po1.tile([128, 512], F32, tag="o1")
            for h in range(4):
                sl = slice(h * 128, (h + 1) * 128)
                nc.tensor.matmul(o1[:, sl], aggTs[:, sl], w1t[:, sl],
                                 start=True, stop=True)
            xo = epool.tile([128, 512], BF16, tag="xo")
            for h in range(2):
                sl = slice(h * 128, (h + 1) * 128)
                nc.vector.scalar_tensor_tensor(
                    xo[:, sl], o1[:, sl], recip[:, h:h + 1], btt[:, sl],
                    AL.mult, AL.add)
            for h in range(2, 4):
                sl = slice(h * 128, (h + 1) * 128)
                nc.scalar.activation(xo[:, sl], o1[:, sl], AF.Copy,
                                     scale=recip[:, h:h + 1])
            nc.vector.tensor_add(xo[:, 256:512], xo[:, 256:512],
                                 btt[:, 256:512])
            nc.sync.dma_start(out[bi * 128:(bi + 1) * 128, :], xo[:])

        prev = front(0)
        for bi in range(1, NBLK):
            cur = front(bi)
            back(bi - 1, *prev)
            prev = cur
        back(NBLK - 1, *prev)
    nc.compile()
    return nc


def _build_proj2():
    """h2sc = elu(x2raw).T-proj: in x2T [512, SHARD] bf16;
    out h2 [SHARD, 256] bf16 + sc [SHARD, 8] f32 (as2|ad2)."""
    nc = bacc.Bacc("TRN2", target_bir_lowering=False, debug=False,
                   num_devices=NCORE)
    xT = nc.dram_tensor("xT", [512, SHARD], BF16, kind="ExternalInput").ap()
    W = nc.dram_tensor("W", [512, 264], BF16, kind="ExternalInput").ap()
    oh = nc.dram_tensor("oh", [SHARD, 256], BF16, kind="ExternalOutput").ap()
    osc = nc.dram_tensor("osc", [SHARD, 8], F32, kind="ExternalOutput").ap()

    with tile.TileContext(nc) as tc, ExitStack() as ctx:
        cpool = ctx.enter_context(tc.tile_pool(name="c", bufs=1))
        pool = ctx.enter_context(tc.tile_pool(name="p", bufs=3))
        epool = ctx.enter_context(tc.tile_pool(name="e", bufs=2))
        psum = ctx.enter_context(tc.tile_pool(name="ps", bufs=2, space="PSUM"))
        wt = cpool.tile([128, 4, 264], BF16)
        for k in range(4):
            nc.sync.dma_start(wt[:, k, :], W[128 * k:128 * (k + 1), :])
        xTr = xT.rearrange("(k p) n -> p k n", p=128)
        GB = 8                                   # blocks of sc staged per DMA
        for bi in range(NBLK):
            xt = pool.tile([128, 4, 128], BF16, tag="xt")
            nc.sync.dma_start(xt[:], xTr[:, :, 128 * bi:(bi + 1) * 128])
            # elu in place: xe = max(exp(min(x,0))-1, x)
            mt = pool.tile([128, 4, 128], BF16, tag="mt")
            nc.vector.tensor_scalar_min(mt[:], xt[:], 0.0)
            nc.scalar.activation(mt[:], mt[:], AF.Exp)
            nc.vector.scalar_tensor_tensor(xt[:], mt[:], 1.0, xt[:],
                                           AL.subtract, AL.max)
            ps = psum.tile([128, 264], F32, tag="ps")
            for k in range(4):
                nc.tensor.matmul(ps[:], xt[:, k, :], wt[:, k, :],
                                 start=(k == 0), stop=(k == 3))
            ht = epool.tile([128, 256], BF16, tag="ht")
            nc.vector.tensor_copy(ht[:, 0:128], ps[:, 0:128])
            nc.scalar.activation(ht[:, 128:256], ps[:, 128:256], AF.Copy)
            nc.sync.dma_start(oh[bi * 128:(bi + 1) * 128, :], ht[:])
            g, j = bi // GB, bi % GB
            if j == 0:
                sct = epool.tile([128, GB, 8], F32, tag="sct")
            nc.vector.tensor_copy(sct[:, j, :], ps[:, 256:264])
            if j == GB - 1 or bi == NBLK - 1:
                nb = j + 1
                oscr = osc.rearrange("(b p) c -> p b c", p=128)
                nc.sync.dma_start(oscr[:, g * GB:g * GB + nb, :],
                                  sct[:, 0:nb, :])
    nc.compile()
    return nc


def _build_agg2(cmax):
    """agg2: x3raw[dst, 256] = (sum alpha h2_src) + b2 (no elu; proj3)."""
    CM = cmax
    nc = bacc.Bacc("TRN2", target_bir_lowering=False, debug=False,
                   num_devices=NCORE)
    fe = nc.dram_tensor("fe", [128, NBLK * CM * 256], BF16,
                        kind="ExternalInput").ap()
    we = nc.dram_tensor("we", [128, NBLK * CM * 8], F32,
                        kind="ExternalInput").ap()
    mk = nc.dram_tensor("mk", [128, NBLK * CM * 128], BF16,
                        kind="ExternalInput").ap()
    bt2 = nc.dram_tensor("bt2", [128, 256], BF16, kind="ExternalInput").ap()
    out = nc.dram_tensor("out", [SHARD, 256], BF16, kind="ExternalOutput").ap()

    with tile.TileContext(nc) as tc, ExitStack() as ctx:
        cpool = ctx.enter_context(tc.tile_pool(name="c", bufs=1))
        spool = ctx.enter_context(tc.tile_pool(name="s", bufs=3))
        vpool = ctx.enter_context(tc.tile_pool(name="v", bufs=2))
        wpool = ctx.enter_context(tc.tile_pool(name="w", bufs=3))
        epool = ctx.enter_context(tc.tile_pool(name="e", bufs=2))
        pagg = ctx.enter_context(tc.tile_pool(name="pa", bufs=3, space="PSUM"))

        WEB = 4
        NACT = 1                              # fs chunks built on ScalarE
        NDVE = CM - NACT
        btt = cpool.tile([128, 256], BF16)
        nc.sync.dma_start(btt[:], bt2[:])

        state = {}

        def front(bi):
            c0 = bi * CM
            if bi % WEB == 0:
                nw = min(WEB, NBLK - bi) * CM
                wet = vpool.tile([128, WEB * CM, 8], F32, tag="wet")
                nc.sync.dma_start(wet[:, 0:nw, :], we[:, c0 * 8:(c0 + nw) * 8])
                zt = vpool.tile([128, WEB * CM, 4], F32, tag="zt")
                nc.vector.tensor_add(zt[:, 0:nw, :], wet[:, 0:nw, 0:4],
                                     wet[:, 0:nw, 4:8])
                nc.vector.scalar_tensor_tensor(zt[:, 0:nw, :], zt[:, 0:nw, :],
                                               NEG, zt[:, 0:nw, :],
                                               AL.mult, AL.max)
                wf = vpool.tile([128, WEB * CM, 4], F32, tag="wf")
                nc.scalar.activation(wf[:, 0:nw, :], zt[:, 0:nw, :], AF.Exp)
                state["wf"] = wf
            wf = state["wf"]
            q = (bi % WEB) * CM
            # fs laid out [128, CM, 264]: cols 0:256 scaled features,
            # cols 256:260 = w (bf16) so one rhs serves agg+den.
            fet = spool.tile([128, CM, 256], BF16, tag="fet")
            nc.sync.dma_start(fet[:], fe[:, c0 * 256:(c0 + CM) * 256])
            mkt = spool.tile([128, CM, 128], BF16, tag="mkt")
            nc.sync.dma_start(mkt[:], mk[:, c0 * 128:(c0 + CM) * 128])

            fs = wpool.tile([128, CM, 264], BF16, tag="fs")
            nc.vector.tensor_copy(fs[:, :, 256:260], wf[:, q:q + CM, :])
            fsv = fs[:, 0:NDVE, 0:256].rearrange("p c (h f) -> p c h f", h=4)
            nc.vector.tensor_tensor(
                fsv,
                fet[:, 0:NDVE, :].rearrange("p c (h f) -> p c h f", h=4),
                wf[:, q:q + NDVE, :].unsqueeze(3).broadcast_to(
                    [128, NDVE, 4, 64]),
                AL.mult)
            for ci in range(NDVE, CM):
                for h in range(4):
                    nc.scalar.activation(fs[:, ci, h * 64:(h + 1) * 64],
                                         fet[:, ci, h * 64:(h + 1) * 64],
                                         AF.Copy, scale=wf[:, q + ci, h:h + 1])

            agg = pagg.tile([128, 260], F32, tag="agg")
            for ci in range(CM):
                nc.tensor.matmul(agg[:], mkt[:, ci, :], fs[:, ci, 0:260],
                                 start=(ci == 0), stop=(ci == CM - 1))
            return (agg,)

        def back(bi, agg):
            recip = epool.tile([128, 4], F32, tag="recip")
            nc.vector.reciprocal(recip[:], agg[:, 256:260])
            xo = epool.tile([128, 256], BF16, tag="xo")
            for h in range(2):
                sl = slice(h * 64, (h + 1) * 64)
                nc.vector.scalar_tensor_tensor(
                    xo[:, sl], agg[:, sl], recip[:, h:h + 1], btt[:, sl],
                    AL.mult, AL.add)
            for h in range(2, 4):
                sl = slice(h * 64, (h + 1) * 64)
                nc.scalar.activation(xo[:, sl], agg[:, sl], AF.Copy,
                                     scale=recip[:, h:h + 1])
            nc.vector.tensor_add(xo[:, 128:256], xo[:, 128:256],
                                 btt[:, 128:256])
            nc.sync.dma_start(out[bi * 128:(bi + 1) * 128, :], xo[:])

        prev = front(0)
        for bi in range(1, NBLK):
            cur = front(bi)
            back(bi - 1, *prev)
            prev = cur
        back(NBLK - 1, *prev)
    nc.compile()
    return nc


def _build_proj3():
    """h3scT = [W3|ws3|wd3].T @ elu(x3raw).T: in x3T [256, SHARD] bf16;
    out [4, SHARD] f32 (h3 2 | as3 | ad3), W stationary."""
    GB = 4
    ngrp = (NBLK + GB - 1) // GB
    nc = bacc.Bacc("TRN2", target_bir_lowering=False, debug=False,
                   num_devices=NCORE)
    xT = nc.dram_tensor("xT", [256, SHARD], BF16, kind="ExternalInput").ap()
    W = nc.dram_tensor("W", [256, 4], BF16, kind="ExternalInput").ap()
    out = nc.dram_tensor("out", [4, SHARD], F32, kind="ExternalOutput").ap()

    with tile.TileContext(nc) as tc, ExitStack() as ctx:
        cpool = ctx.enter_context(tc.tile_pool(name="c", bufs=1))
        pool = ctx.enter_context(tc.tile_pool(name="p", bufs=3))
        psum = ctx.enter_context(tc.tile_pool(name="ps", bufs=2, space="PSUM"))
        wt = cpool.tile([128, 2, 4], BF16)
        for k in range(2):
            nc.sync.dma_start(wt[:, k, :], W[128 * k:128 * (k + 1), :])
        xTr = xT.rearrange("(k p) n -> p k n", p=128)
        for g in range(ngrp):
            nb = min(GB, NBLK - g * GB)
            xt = pool.tile([128, 2, GB, 128], BF16, tag="xt")
            nc.sync.dma_start(xt[:, :, 0:nb, :],
                              xTr[:, :, g * GB * 128:(g * GB + nb) * 128]
                              .rearrange("p k (b n) -> p k b n", n=128))
            mt = pool.tile([128, 2, GB, 128], BF16, tag="mt")
            nc.vector.tensor_scalar_min(mt[:, :, 0:nb, :], xt[:, :, 0:nb, :],
                                        0.0)
            nc.scalar.activation(mt[:, :, 0:nb, :], mt[:, :, 0:nb, :], AF.Exp)
            nc.vector.scalar_tensor_tensor(xt[:, :, 0:nb, :], mt[:, :, 0:nb, :],
                                           1.0, xt[:, :, 0:nb, :],
                                           AL.subtract, AL.max)
            ps = psum.tile([4, GB * 128], F32, tag="ps")
            for j in range(nb):
                for k in range(2):
                    nc.tensor.matmul(ps[:, j * 128:(j + 1) * 128],
                                     wt[:, k, :], xt[:, k, j, :],
                                     start=(k == 0), stop=(k == 1))
            ot = pool.tile([4, GB * 128], F32, tag="ot")
            nc.vector.tensor_copy(ot[:, 0:nb * 128], ps[:, 0:nb * 128])
            nc.sync.dma_start(out[:, g * GB * 128:g * GB * 128 + nb * 128],
                              ot[:, 0:nb * 128])
    nc.compile()
    return nc


def _build_agg3(cmax):
    """agg3: out[dst, 2] = (sum alpha h3_src) + b3, heads=1."""
    CM = cmax
    SB = 4
    nc = bacc.Bacc("TRN2", target_bir_lowering=False, debug=False,
                   num_devices=NCORE)
    fe = nc.dram_tensor("fe", [128, NBLK * CM * 2], BF16,
                        kind="ExternalInput").ap()
    we = nc.dram_tensor("we", [128, NBLK * CM * 2], F32,
                        kind="ExternalInput").ap()
    mk = nc.dram_tensor("mk", [128, NBLK * CM * 128], BF16,
                        kind="ExternalInput").ap()
    bt3 = nc.dram_tensor("bt3", [128, 2], F32, kind="ExternalInput").ap()
    out = nc.dram_tensor("out", [SHARD, 2], F32, kind="ExternalOutput").ap()

    with tile.TileContext(nc) as tc, ExitStack() as ctx:
        cpool = ctx.enter_context(tc.tile_pool(name="c", bufs=1))
        spool = ctx.enter_context(tc.tile_pool(name="s", bufs=3))
        wpool = ctx.enter_context(tc.tile_pool(name="w", bufs=2))
        epool = ctx.enter_context(tc.tile_pool(name="e", bufs=2))
        pagg = ctx.enter_context(tc.tile_pool(name="pa", bufs=3, space="PSUM"))

        btt = cpool.tile([128, 2], F32)
        nc.sync.dma_start(btt[:], bt3[:])

        nsb = NBLK // SB + (1 if NBLK % SB else 0)
        for sb in range(nsb):
            b0 = sb * SB
            nb = min(SB, NBLK - b0)
            c0 = b0 * CM
            nch = nb * CM
            fet = spool.tile([128, SB * CM, 2], BF16, tag="fet")
            nc.sync.dma_start(fet[:, 0:nch, :], fe[:, c0 * 2:(c0 + nch) * 2])
            wet = spool.tile([128, SB * CM, 2], F32, tag="wet")
            nc.sync.dma_start(wet[:, 0:nch, :], we[:, c0 * 2:(c0 + nch) * 2])
            mkt = spool.tile([128, SB * CM, 128], BF16, tag="mkt")
            nc.sync.dma_start(mkt[:, 0:nch, :],
                              mk[:, c0 * 128:(c0 + nch) * 128])

            zt = wpool.tile([128, SB * CM, 1], F32, tag="zt")
            nc.vector.tensor_add(zt[:, 0:nch, :], wet[:, 0:nch, 0:1],
                                 wet[:, 0:nch, 1:2])
            nc.vector.scalar_tensor_tensor(zt[:, 0:nch, :], zt[:, 0:nch, :],
                                           NEG, zt[:, 0:nch, :],
                                           AL.mult, AL.max)
            wf = wpool.tile([128, SB * CM, 1], F32, tag="wf")
            nc.scalar.activation(wf[:, 0:nch, :], zt[:, 0:nch, :], AF.Exp)
            rhs = wpool.tile([128, SB * CM, 4], BF16, tag="rhs")
            nc.vector.tensor_tensor(
                rhs[:, 0:nch, 0:2], fet[:, 0:nch, :],
                wf[:, 0:nch, :].broadcast_to([128, nch, 2]), AL.mult)
            nc.vector.tensor_copy(rhs[:, 0:nch, 2:3], wf[:, 0:nch, :])

            xs = epool.tile([128, SB, 2], F32, tag="xs")
            for b in range(nb):
                agg = pagg.tile([128, 3], F32, tag="agg")
                for ci in range(CM):
                    cc = b * CM + ci
                    nc.tensor.matmul(agg[:], mkt[:, cc, :], rhs[:, cc, 0:3],
                                     start=(ci == 0), stop=(ci == CM - 1))
                recip = epool.tile([128, 1], F32, tag="recip")
                nc.vector.reciprocal(recip[:], agg[:, 2:3])
                nc.vector.scalar_tensor_tensor(
                    xs[:, b, :], agg[:, 0:2], recip[:], btt[:],
                    AL.mult, AL.add)
            outr = out.rearrange("(b p) c -> p b c", p=128)
            nc.sync.dma_start(outr[:, b0:b0 + nb, :], xs[:, 0:nb, :])
    nc.compile()
    return nc


# --------------------------------------------------------------------------
# orchestration
# --------------------------------------------------------------------------

def _get_program(key, builder):
    if key not in _program_cache:
        _program_cache[key] = builder()
    return _program_cache[key]


def _run(stage, nc, in_maps):
    if TRACE:
        _install_profile_shim()
    res = run_bass_kernel_spmd(nc, in_maps, core_ids=list(range(NCORE)),
                               trace=TRACE, trace_cores=list(range(NCORE)),
                               stitch_traces=False)
    if res.exec_time_ns is not None:
        _last_stage_times[stage] = res.exec_time_ns
    return res


def _fold_ws(W, a, heads, out_c):
    Wr = W.reshape(W.shape[0], heads, out_c)
    return np.einsum('fhc,hc->fh', Wr.astype(np.float64),
                     a.astype(np.float64)).astype(np.float32)


def _bcast(b, outc, dtype):
    return np.ascontiguousarray(
        np.broadcast_to(np.asarray(b, np.float32).reshape(1, outc),
                        (128, outc)).astype(dtype))


def _pad_tab(tab):
    """[N, C] node table -> [NPAD+1, C] with zero pad rows (row N..: zeros)."""
    out = np.zeros((NPAD + 1, tab.shape[1]), tab.dtype)
    out[:tab.shape[0]] = tab
    return out


def kernel(x, edge_index, W1, a_src1, a_dst1, b1, W2, a_src2, a_dst2, b2,
           W3, a_src3, a_dst3, b3):
    x = np.asarray(x, np.float32)
    W1 = np.asarray(W1, np.float32); W2 = np.asarray(W2, np.float32)
    W3 = np.asarray(W3, np.float32)

    cores, cmax, slot_of_node, node_of_slot = _plan(np.asarray(edge_index))

    ws1 = _fold_ws(W1, np.asarray(a_src1, np.float32), 4, 128)
    wd1 = _fold_ws(W1, np.asarray(a_dst1, np.float32), 4, 128)
    ws2 = _fold_ws(W2, np.asarray(a_src2, np.float32), 4, 64)
    wd2 = _fold_ws(W2, np.asarray(a_dst2, np.float32), 4, 64)
    ws3 = _fold_ws(W3, np.asarray(a_src3, np.float32), 1, 2)
    wd3 = _fold_ws(W3, np.asarray(a_dst3, np.float32), 1, 2)

    # node features in slot order (xs[slot] = x[node_of_slot[slot]])
    xs = np.zeros((NPAD, 128), np.float32)
    real = node_of_slot < N
    xs[real] = x[node_of_slot[real]]

    masks = [_mask_stream(c) for c in cores]

    # stage 1: as1/ad1 for all slots (shard-local rows)
    s1 = _get_program("proj1", _build_proj1)
    wsd1 = np.concatenate([ws1, wd1], axis=1)
    r1 = _run("proj1", s1, [
        {"xT": np.ascontiguousarray(xs[c * SHARD:(c + 1) * SHARD].T),
         "W": wsd1} for c in range(NCORE)])
    asad1 = np.concatenate([r1.results[c]["out"].T for c in range(NCORE)])
    # as1 by node id: asad1 is slot-indexed
    as1_node = np.zeros((NPAD + 1, 4), np.float32)
    as1_node[node_of_slot] = asad1[:, 0:4]
    ad1_slot = asad1[:, 4:8]

    # stage 2: agg1 + W1 proj -> x2raw (slot-major [NPAD, 512] bf16)
    xtab = np.zeros((NPAD + 1, 128), BF)       # node-indexed feature table
    xtab[:N] = x.astype(BF)
    s2 = _get_program(("agg1", cmax), lambda: _build_agg1(cmax))
    in2 = [{"fe": _stream(cores[c], xtab, BF),
            "we": _we_stream(cores[c], as1_node, ad1_slot, c, cmax),
            "mk": masks[c],
            "W1b": W1.astype(BF),
            "bt1": _bcast(b1, 512, BF)} for c in range(NCORE)]
    r2 = _run("agg1", s2, in2)
    x2 = np.concatenate([r2.results[c]["out"] for c in range(NCORE)])

    # stage 3: proj2 (elu + W2|ws2|wd2)
    s3 = _get_program("proj2", _build_proj2)
    W2s = np.concatenate([W2, ws2, wd2], axis=1).astype(BF)
    r3 = _run("proj2", s3, [
        {"xT": np.ascontiguousarray(x2[c * SHARD:(c + 1) * SHARD].T),
         "W": W2s} for c in range(NCORE)])
    h2 = np.concatenate([r3.results[c]["oh"] for c in range(NCORE)])
    sc2 = np.concatenate([r3.results[c]["osc"] for c in range(NCORE)])
    as2_node = np.zeros((NPAD + 1, 4), np.float32)
    as2_node[node_of_slot] = sc2[:, 0:4]
    ad2_slot = sc2[:, 4:8]

    # stage 4: agg2 -> x3raw
    h2tab = np.zeros((NPAD + 1, 256), BF)
    h2tab[node_of_slot] = h2
    s4 = _get_program(("agg2", cmax), lambda: _build_agg2(cmax))
    in4 = [{"fe": _stream(cores[c], h2tab, BF),
            "we": _we_stream(cores[c], as2_node, ad2_slot, c, cmax),
            "mk": masks[c],
            "bt2": _bcast(b2, 256, BF)} for c in range(NCORE)]
    r4 = _run("agg2", s4, in4)
    x3 = np.concatenate([r4.results[c]["out"] for c in range(NCORE)])

    # stage 5: proj3 (elu + W3|ws3|wd3), W stationary, transposed out
    s5 = _get_program("proj3", _build_proj3)
    W3s = np.concatenate([W3, ws3, wd3], axis=1).astype(BF)
    r5 = _run("proj3", s5, [
        {"xT": np.ascontiguousarray(x3[c * SHARD:(c + 1) * SHARD].T),
         "W": W3s} for c in range(NCORE)])
    h3sc = np.concatenate([r5.results[c]["out"].T for c in range(NCORE)])
    h3_slot, as3_slot, ad3_slot = (h3sc[:, 0:2], h3sc[:, 2:3], h3sc[:, 3:4])
    h3tab = np.zeros((NPAD + 1, 2), BF)
    h3tab[node_of_slot] = h3_slot
    as3_node = np.zeros((NPAD + 1, 1), np.float32)
    as3_node[node_of_slot] = as3_slot

    # stage 6: agg3 -> out
    s6 = _get_program(("agg3", cmax), lambda: _build_agg3(cmax))
    in6 = [{"fe": _stream(cores[c], h3tab, BF),
            "we": _we_stream(cores[c], as3_node, ad3_slot, c, cmax),
            "mk": masks[c],
            "bt3": _bcast(b3, 2, np.float32)} for c in range(NCORE)]
    r6 = _run("agg3", s6, in6)
    outp = np.concatenate([r6.results[c]["out"] for c in range(NCORE)])
    return np.ascontiguousarray(outp[slot_of_node[:N]]).astype(np.float32)


# revision 27
# speedup vs baseline: 4.1316x; 1.1741x over previous
"""3-layer GAT (PyG GATConv-style) on 8 Trainium2 NeuronCores — v2.

Strategy (dst-node sharding, all fp math on device):
  - Nodes are permuted into 100352 slots = 8 shards x 98 blocks x 128 via an
    LPT bin-pack on in-degree so every block has ~equal incoming-edge count
    (cmax = ceil(max_block_edges/128) drops 6 -> 5).
  - Edges (incl. self-loops) are partitioned by dst block; per (core, block)
    the edge list is padded to cmax 128-edge chunks; all 8 cores run one SPMD
    program per stage.
  - Segment softmax + weighted segment-sum run on TensorE via HOST-BUILT
    one-hot dst masks (plain DMA; no device gather).  The softmax max-shift
    is dropped (shift invariance; logits are O(10), safe in fp32 exp range);
    the denominator comes from a mask.T @ w matmul and is applied at PSUM
    evacuation.  Self-loops guarantee den > 0, so no epsilon term.
  - 6 SPMD stages: proj1 (as/ad), agg1(+W1 proj), proj2 (elu+W2|as2|ad2),
    agg2, proj3 (elu+W3|as3|ad3), agg3.  Host glue does layout only
    (permutation, row gathers by edge index, transpose, pad).
"""
import sys

sys.path.insert(0, "/opt/trn_rl_repo")

import heapq
import numpy as np
import ml_dtypes
from contextlib import ExitStack

import concourse.bass as bass
import concourse.bacc as bacc
import concourse.tile as tile
import concourse.mybir as mybir
from concourse.bass_utils import run_bass_kernel_spmd

F32 = mybir.dt.float32
F32R = mybir.dt.float32r
BF16 = mybir.dt.bfloat16
AL = mybir.AluOpType
AF = mybir.ActivationFunctionType
BF = ml_dtypes.bfloat16

N = 100000
NPAD = 100352            # 8 * 98 * 128
NCORE = 8
SHARD = NPAD // NCORE    # 12544
NBLK = SHARD // 128      # 98
NBLK_ALL = NPAD // 128   # 784
NEG = 0.2

_program_cache = {}
_last_stage_times = {}   # stage -> exec_time_ns (filled when trace=True)
TRACE = False
_shim_done = [False]


def _install_profile_shim():
    """The agent image's antenv lacks axon_hooks; recreate the tiny shim so
    run_bass_kernel_spmd(trace=True) can drive NTFF profiling via the axon
    plugin's C ABI, and stub the S3 artifact upload (no creds here)."""
    if _shim_done[0]:
        return
    import types
    mod = types.ModuleType("antenv.axon_hooks")
    holder = [None]
    mod.set_axon_ntff_profile_hook = lambda h: holder.__setitem__(0, h)
    mod.get_axon_ntff_profile_hook = lambda: holder[0]
    sys.modules["antenv.axon_hooks"] = mod
    from trn_agent_boot.trn_boot import _ntff_profile_via_ctypes
    holder[0] = _ntff_profile_via_ctypes('/opt/axon/libaxon_pjrt.so')
    import concourse.bass_utils as bu
    bu.upload_artifacts = lambda tmpdir: "local://" + str(tmpdir)
    _shim_done[0] = True


# --------------------------------------------------------------------------
# host-side preprocessing (indices / layout only)
# --------------------------------------------------------------------------

def _balance_slots(deg):
    """LPT bin-pack: assign nodes (by desc in-degree) to 784 blocks of 128
    slots so block edge counts are ~equal.  Returns slot_of_node[NPAD]."""
    order = np.argsort(-deg, kind="stable")
    heap = [(0, 0, b) for b in range(NBLK_ALL)]
    heapq.heapify(heap)
    slot_of_node = np.empty(NPAD, np.int64)
    fill = np.zeros(NBLK_ALL, np.int32)
    # process in runs of equal degree: round-robin via heap
    for n in order:
        s, c, b = heapq.heappop(heap)
        slot_of_node[n] = b * 128 + fill[b]
        fill[b] += 1
        c += 1
        if c < 128:
            heapq.heappush(heap, (s + int(deg[n]), c, b))
    return slot_of_node


def _plan(edge_index):
    src = np.concatenate([np.asarray(edge_index[0], np.int64),
                          np.arange(N, dtype=np.int64)])
    dst = np.concatenate([np.asarray(edge_index[1], np.int64),
                          np.arange(N, dtype=np.int64)])
    deg = np.bincount(dst, minlength=NPAD)  # padded "nodes" N..NPAD-1: deg 0
    slot_of_node = _balance_slots(deg)
    node_of_slot = np.empty(NPAD, np.int64)
    node_of_slot[slot_of_node] = np.arange(NPAD)

    dslot = slot_of_node[dst]
    blk = dslot // 128
    order = np.argsort(blk * 256 + (dslot % 128) // 64, kind="stable")
    src, dslot, blk = src[order], dslot[order], blk[order]
    bc = np.bincount(blk, minlength=NBLK_ALL)
    cmax = int(np.ceil(bc.max() / 128))
    starts = np.zeros(NBLK_ALL + 1, np.int64)
    np.cumsum(bc, out=starts[1:])
    L = cmax * 128

    # token slot for each edge: block-local position + block base
    within = np.arange(len(src)) - starts[blk]
    tok = blk * L + within                      # global padded token index
    T_all = NBLK_ALL * L
    tok_src = np.full(T_all, N, np.int64)       # N -> zero row in tables
    tok_dstl = np.full(T_all, 0, np.int64)
    tok_valid = np.zeros(T_all, bool)
    tok_src[tok] = src
    tok_dstl[tok] = dslot % 128
    tok_valid[tok] = True

    Tc = NBLK * L                               # tokens per core
    cores = []
    for c in range(NCORE):
        sl = slice(c * Tc, (c + 1) * Tc)
        cores.append(dict(src=tok_src[sl], dstl=tok_dstl[sl],
                          valid=tok_valid[sl]))
    return cores, cmax, slot_of_node, node_of_slot


def _chunkmaj(rows):
    """[T, C] row-major -> [128, T/128 * C] token-partition-major."""
    Tn, C = rows.shape
    ch = rows.reshape(Tn // 128, 128, C).transpose(1, 0, 2).reshape(128, -1)
    return np.ascontiguousarray(ch)


def _stream(core_plan, table, dtype):
    """Host row-gather by token src id -> [128, T/128 * C]."""
    Tn = core_plan["src"].shape[0]
    C = table.shape[1]
    rows = np.zeros((Tn, C), dtype)
    v = core_plan["valid"]
    rows[v] = table[core_plan["src"][v]].astype(dtype)
    return _chunkmaj(rows)


def _fm_stream(core_plan, table, pad=0):
    """Host [gathered-features | one-hot mask | pad] -> [128, T/128*W] bf16."""
    Tn = core_plan["src"].shape[0]
    C = table.shape[1]
    rows = np.zeros((Tn, C + 128 + pad), BF)
    v = np.nonzero(core_plan["valid"])[0]
    rows[v, 0:C] = table[core_plan["src"][v]]
    rows[v, C + core_plan["dstl"][v]] = 1
    return _chunkmaj(rows)


def _we_stream(core_plan, src_tab, dst_tab, core_id, cmax):
    """[as(src) | ad(dst)] per token -> [128, T/128 * 2H] f32.
    src_tab/dst_tab are [NPAD(+1), H] node-indexed (row N.. = zeros)."""
    Tn = core_plan["src"].shape[0]
    H = src_tab.shape[1]
    rows = np.zeros((Tn, 2 * H), np.float32)
    v = core_plan["valid"]
    rows[v, :H] = src_tab[core_plan["src"][v]]
    L = cmax * 128
    blkl = np.arange(Tn) // L
    dst_slot = (core_id * NBLK + blkl) * 128 + core_plan["dstl"]
    rows[v, H:] = dst_tab[dst_slot[v]]          # dst_tab slot-indexed
    return _chunkmaj(rows)


# --------------------------------------------------------------------------
# stage program builders
# --------------------------------------------------------------------------

def _build_proj1():
    """asadT[8, SHARD] = ([ws1|wd1].T @ xT) with W stationary."""
    GB = 7                                   # blocks per group (98 = 14 x 7)
    ngrp = NBLK // GB
    nc = bacc.Bacc("TRN2", target_bir_lowering=False, debug=False,
                   num_devices=NCORE)
    xT = nc.dram_tensor("xT", [128, SHARD], F32, kind="ExternalInput").ap()
    W = nc.dram_tensor("W", [128, 8], F32, kind="ExternalInput").ap()
    out = nc.dram_tensor("out", [8, SHARD], F32, kind="ExternalOutput").ap()

    with tile.TileContext(nc) as tc, ExitStack() as ctx:
        cpool = ctx.enter_context(tc.tile_pool(name="c", bufs=1))
        pool = ctx.enter_context(tc.tile_pool(name="p", bufs=3))
        epool = ctx.enter_context(tc.tile_pool(name="e", bufs=3))
        psum = ctx.enter_context(tc.tile_pool(name="ps", bufs=2, space="PSUM"))
        wt = cpool.tile([128, 8], F32)
        nc.sync.dma_start(wt[:], W[:])
        xTr = xT.rearrange("p (b n) -> p b n", n=128)
        for g in range(ngrp):
            xt = pool.tile([128, GB, 128], F32, tag="xt")
            nc.sync.dma_start(xt[:], xTr[:, g * GB:(g + 1) * GB, :])
            ps = psum.tile([8, GB * 128], F32, tag="ps")
            for j in range(GB):
                nc.tensor.matmul(ps[:, j * 128:(j + 1) * 128],
                                 wt[:], xt[:, j, :],
                                 start=True, stop=True)
            ot = epool.tile([8, GB * 128], F32, tag="ot")
            nc.vector.tensor_copy(ot[:], ps[:])
            nc.gpsimd.dma_start(out[:, g * GB * 128:(g + 1) * GB * 128],
                                ot[:])
    nc.compile()
    return nc


def _build_agg1(cmax):
    """agg1 + W1 proj: x2raw[dst, 512] = (sum alpha x_src) @ W1 + b1
    (no elu here; proj2 applies it).  Software-pipelined: heavy PSUM-side
    work of block i is emitted after the stream work of block i+1."""
    CM = cmax
    WEB = 4                                   # blocks per we-DMA batch
    SB = 2                                    # blocks per stream-DMA batch
    NACT = max(1, CM // 4)                    # wm chunks built on ScalarE
    NDVE = CM - NACT                          # wm chunks built on VectorE
    W = 256                                   # stream width: fe 128 | mask 128
    nc = bacc.Bacc("TRN2", target_bir_lowering=False, debug=False,
                   num_devices=NCORE)
    fm = nc.dram_tensor("fm", [128, NBLK * CM * W], BF16,
                        kind="ExternalInput").ap()
    we = nc.dram_tensor("we", [128, NBLK * CM * 8], F32,
                        kind="ExternalInput").ap()
    W1b = nc.dram_tensor("W1b", [128, 512], BF16, kind="ExternalInput").ap()
    bt1 = nc.dram_tensor("bt1", [128, 512], BF16, kind="ExternalInput").ap()
    out = nc.dram_tensor("out", [SHARD, 512], BF16, kind="ExternalOutput").ap()

    with tile.TileContext(nc) as tc, ExitStack() as ctx:
        cpool = ctx.enter_context(tc.tile_pool(name="c", bufs=1))
        spool = ctx.enter_context(tc.tile_pool(name="s", bufs=3))
        vpool = ctx.enter_context(tc.tile_pool(name="v", bufs=2))
        wpool = ctx.enter_context(tc.tile_pool(name="w", bufs=3))
        epool = ctx.enter_context(tc.tile_pool(name="e", bufs=2))
        opool = ctx.enter_context(tc.tile_pool(name="o", bufs=2))
        pagg = ctx.enter_context(tc.tile_pool(name="pa", bufs=2, space="PSUM"))
        pden = ctx.enter_context(tc.tile_pool(name="pd", bufs=2, space="PSUM"))
        po1 = ctx.enter_context(tc.tile_pool(name="po", bufs=2, space="PSUM"))

        w1t = cpool.tile([128, 512], BF16)
        nc.sync.dma_start(w1t[:], W1b[:])
        btt = cpool.tile([128, 512], BF16)
        nc.sync.dma_start(btt[:], bt1[:])

        state = {}

        def front(bi):
            c0 = bi * CM
            if bi % WEB == 0:
                nw = min(WEB, NBLK - bi) * CM
                wet = vpool.tile([128, WEB * CM, 8], F32, tag="wet")
                nc.sync.dma_start(wet[:, 0:nw, :], we[:, c0 * 8:(c0 + nw) * 8])
                zt = vpool.tile([128, WEB * CM, 4], F32, tag="zt")
                nc.vector.tensor_add(zt[:, 0:nw, :], wet[:, 0:nw, 0:4],
                                     wet[:, 0:nw, 4:8])
                nc.vector.scalar_tensor_tensor(zt[:, 0:nw, :], zt[:, 0:nw, :],
                                               NEG, zt[:, 0:nw, :],
                                               AL.mult, AL.max)
                wf = vpool.tile([128, WEB * CM, 4], F32, tag="wf")
                nc.scalar.activation(wf[:, 0:nw, :], zt[:, 0:nw, :], AF.Exp)
                wb = vpool.tile([128, WEB * CM, 4], BF16, tag="wb")
                nc.vector.tensor_copy(wb[:, 0:nw, :], wf[:, 0:nw, :])
                state["wf"], state["wb"] = wf, wb
            if bi % SB == 0:
                ns = min(SB, NBLK - bi) * CM
                st = spool.tile([128, SB * CM, W], BF16, tag="st")
                nc.sync.dma_start(st[:, 0:ns, :], fm[:, c0 * W:(c0 + ns) * W])
                state["st"] = st
            wf, wb = state["wf"], state["wb"]
            st = state["st"]
            q = (bi % WEB) * CM
            s = (bi % SB) * CM
            fet = st[:, s:s + CM, 0:128]
            mkt = st[:, s:s + CM, 128:256]

            wm = wpool.tile([128, CM, 4, 128], BF16, tag="wm")
            nc.vector.tensor_tensor(
                wm[:, 0:NDVE, :, :],
                st[:, s:s + NDVE, 128:256].unsqueeze(2).broadcast_to(
                    [128, NDVE, 4, 128]),
                wf[:, q:q + NDVE, :].unsqueeze(3).broadcast_to(
                    [128, NDVE, 4, 128]),
                AL.mult)
            for ci in range(NDVE, CM):
                for h in range(4):
                    nc.scalar.activation(wm[:, ci, h, :],
                                         st[:, s + ci, 128:256],
                                         AF.Copy, scale=wf[:, q + ci, h:h + 1])

            aggT = pagg.tile([128, 512], F32, tag="aggT")
            den = pden.tile([128, 4], F32, tag="den")
            for ci in range(CM):
                nc.tensor.matmul(
                    aggT[:], fet[:, ci, :],
                    wm[:, ci, :, :].rearrange("p h d -> p (h d)"),
                    start=(ci == 0), stop=(ci == CM - 1))
            for ci in range(CM):
                nc.tensor.matmul(den[:], mkt[:, ci, :], wb[:, q + ci, :],
                                 start=(ci == 0), stop=(ci == CM - 1),
                                 skip_group_check=True)
            return aggT, den

        def back(bi, aggT, den):
            recip = epool.tile([128, 4], F32, tag="recip")
            nc.vector.reciprocal(recip[:], den[:])
            aggTs = epool.tile([128, 512], BF16, tag="aggTs")
            nc.vector.tensor_copy(aggTs[:, 0:256], aggT[:, 0:256])
            nc.scalar.activation(aggTs[:, 256:512], aggT[:, 256:512], AF.Copy)
            o1 = po1.tile([128, 512], F32, tag="o1")
            for h in range(4):
                sl = slice(h * 128, (h + 1) * 128)
                nc.tensor.matmul(o1[:, sl], aggTs[:, sl], w1t[:, sl],
                                 start=True, stop=True)
            if bi % SB == 0:
                state["xo"] = opool.tile([128, SB, 512], BF16, tag="xo", name="xo")
            xo = state["xo"][:, bi % SB, :]
            for h in range(2):
                sl = slice(h * 128, (h + 1) * 128)
                nc.vector.scalar_tensor_tensor(
                    xo[:, sl], o1[:, sl], recip[:, h:h + 1], btt[:, sl],
                    AL.mult, AL.add)
            for h in range(2, 4):
                sl = slice(h * 128, (h + 1) * 128)
                nc.scalar.activation(xo[:, sl], o1[:, sl], AF.Copy,
                                     scale=recip[:, h:h + 1])
            nc.vector.tensor_add(xo[:, 256:512], xo[:, 256:512],
                                 btt[:, 256:512])
            if bi % SB == SB - 1 or bi == NBLK - 1:
                b0 = bi - bi % SB
                nb = bi % SB + 1
                nc.gpsimd.dma_start(
                    out.rearrange("(b p) c -> p b c", p=128)[:, b0:b0 + nb, :],
                    state["xo"][:, 0:nb, :])

        prev = front(0)
        for bi in range(1, NBLK):
            cur = front(bi)
            back(bi - 1, *prev)
            prev = cur
        back(NBLK - 1, *prev)
    nc.compile()
    return nc


def _build_proj2():
    """h2sc = elu(x2raw).T-proj: in x2T [512, SHARD] bf16;
    out h2 [SHARD, 256] bf16 + sc [SHARD, 8] f32 (as2|ad2)."""
    nc = bacc.Bacc("TRN2", target_bir_lowering=False, debug=False,
                   num_devices=NCORE)
    xT = nc.dram_tensor("xT", [512, SHARD], BF16, kind="ExternalInput").ap()
    W = nc.dram_tensor("W", [512, 264], BF16, kind="ExternalInput").ap()
    oh = nc.dram_tensor("oh", [SHARD, 256], BF16, kind="ExternalOutput").ap()
    osc = nc.dram_tensor("osc", [SHARD, 8], F32, kind="ExternalOutput").ap()

    with tile.TileContext(nc) as tc, ExitStack() as ctx:
        cpool = ctx.enter_context(tc.tile_pool(name="c", bufs=1))
        pool = ctx.enter_context(tc.tile_pool(name="p", bufs=3))
        epool = ctx.enter_context(tc.tile_pool(name="e", bufs=2))
        psum = ctx.enter_context(tc.tile_pool(name="ps", bufs=3, space="PSUM"))
        SB = 2
        GB = 8                                   # blocks of sc staged per DMA
        wt = cpool.tile([128, 4, 264], BF16)
        for k in range(4):
            nc.sync.dma_start(wt[:, k, :], W[128 * k:128 * (k + 1), :])
        xTr = xT.rearrange("(k p) n -> p k n", p=128)
        state = {}

        def front(bi):
            if bi % SB == 0:
                n = min(SB, NBLK - bi) * 128
                xt = pool.tile([128, 4, SB * 128], BF16, tag="xt")
                nc.sync.dma_start(xt[:, :, 0:n],
                                  xTr[:, :, 128 * bi:128 * bi + n])
                # elu in place: xe = max(exp(min(x,0))-1, x)
                mt = pool.tile([128, 4, SB * 128], BF16, tag="mt")
                nc.vector.tensor_scalar_min(mt[:, :, 0:n], xt[:, :, 0:n], 0.0)
                nc.scalar.activation(mt[:, :, 0:n], mt[:, :, 0:n], AF.Exp)
                nc.vector.scalar_tensor_tensor(xt[:, :, 0:n], mt[:, :, 0:n],
                                               1.0, xt[:, :, 0:n],
                                               AL.subtract, AL.max)
                state["xt"] = xt
            xt = state["xt"]
            s = (bi % SB) * 128
            ps = psum.tile([128, 264], F32, tag="ps")
            for k in range(4):
                nc.tensor.matmul(ps[:], xt[:, k, s:s + 128], wt[:, k, :],
                                 start=(k == 0), stop=(k == 3))
            return (ps,)

        def back(bi, ps):
            if bi % SB == 0:
                state["ht"] = epool.tile([128, SB, 256], BF16, tag="ht", name="ht")
            ht = state["ht"][:, bi % SB, :]
            nc.vector.tensor_copy(ht[:, 0:128], ps[:, 0:128])
            nc.scalar.activation(ht[:, 128:256], ps[:, 128:256], AF.Copy)
            if bi % SB == SB - 1 or bi == NBLK - 1:
                b0 = bi - bi % SB
                nb = bi % SB + 1
                nc.gpsimd.dma_start(
                    oh.rearrange("(b p) c -> p b c", p=128)[:, b0:b0 + nb, :],
                    state["ht"][:, 0:nb, :])
            g, j = bi // GB, bi % GB
            if j == 0:
                state["sct"] = epool.tile([128, GB, 8], F32, tag="sct", name="sct")
            nc.vector.tensor_copy(state["sct"][:, j, :], ps[:, 256:264])
            if j == GB - 1 or bi == NBLK - 1:
                nb = j + 1
                oscr = osc.rearrange("(b p) c -> p b c", p=128)
                nc.gpsimd.dma_start(oscr[:, g * GB:g * GB + nb, :],
                                    state["sct"][:, 0:nb, :])

        prev = front(0)
        for bi in range(1, NBLK):
            cur = front(bi)
            back(bi - 1, *prev)
            prev = cur
        back(NBLK - 1, *prev)
    nc.compile()
    return nc


def _build_agg2(cmax):
    """agg2: x3raw[dst, 256] = (sum alpha h2_src) + b2 (no elu; proj3)."""
    CM = cmax
    W = 384                                   # fe 256 | mask 128
    nc = bacc.Bacc("TRN2", target_bir_lowering=False, debug=False,
                   num_devices=NCORE)
    fm = nc.dram_tensor("fm", [128, NBLK * CM * W], BF16,
                        kind="ExternalInput").ap()
    we = nc.dram_tensor("we", [128, NBLK * CM * 8], F32,
                        kind="ExternalInput").ap()
    bt2 = nc.dram_tensor("bt2", [128, 256], BF16, kind="ExternalInput").ap()
    out = nc.dram_tensor("out", [SHARD, 256], BF16, kind="ExternalOutput").ap()

    with tile.TileContext(nc) as tc, ExitStack() as ctx:
        cpool = ctx.enter_context(tc.tile_pool(name="c", bufs=1))
        spool = ctx.enter_context(tc.tile_pool(name="s", bufs=3))
        vpool = ctx.enter_context(tc.tile_pool(name="v", bufs=2))
        wpool = ctx.enter_context(tc.tile_pool(name="w", bufs=3))
        epool = ctx.enter_context(tc.tile_pool(name="e", bufs=2))
        opool = ctx.enter_context(tc.tile_pool(name="o", bufs=2))
        pagg = ctx.enter_context(tc.tile_pool(name="pa", bufs=3, space="PSUM"))

        WEB = 4
        SB = 2
        NACT = 1                              # fs chunks built on ScalarE
        NDVE = CM - NACT
        btt = cpool.tile([128, 256], BF16)
        nc.sync.dma_start(btt[:], bt2[:])

        state = {}

        def front(bi):
            c0 = bi * CM
            if bi % WEB == 0:
                nw = min(WEB, NBLK - bi) * CM
                wet = vpool.tile([128, WEB * CM, 8], F32, tag="wet")
                nc.sync.dma_start(wet[:, 0:nw, :], we[:, c0 * 8:(c0 + nw) * 8])
                zt = vpool.tile([128, WEB * CM, 4], F32, tag="zt")
                nc.vector.tensor_add(zt[:, 0:nw, :], wet[:, 0:nw, 0:4],
                                     wet[:, 0:nw, 4:8])
                nc.vector.scalar_tensor_tensor(zt[:, 0:nw, :], zt[:, 0:nw, :],
                                               NEG, zt[:, 0:nw, :],
                                               AL.mult, AL.max)
                wf = vpool.tile([128, WEB * CM, 4], F32, tag="wf")
                nc.scalar.activation(wf[:, 0:nw, :], zt[:, 0:nw, :], AF.Exp)
                state["wf"] = wf
            if bi % SB == 0:
                ns = min(SB, NBLK - bi) * CM
                st = spool.tile([128, SB * CM, W], BF16, tag="st")
                nc.sync.dma_start(st[:, 0:ns, :], fm[:, c0 * W:(c0 + ns) * W])
                state["st"] = st
            wf = state["wf"]
            st = state["st"]
            q = (bi % WEB) * CM
            s = (bi % SB) * CM
            mkt = st[:, s:s + CM, 256:384]
            # fs laid out [128, CM, 264]: cols 0:256 scaled features,
            # cols 256:260 = w (bf16) so one rhs serves agg+den.
            fs = wpool.tile([128, CM, 264], BF16, tag="fs")
            nc.vector.tensor_copy(fs[:, :, 256:260], wf[:, q:q + CM, :])
            fsv = fs[:, 0:NDVE, 0:256].rearrange("p c (h f) -> p c h f", h=4)
            nc.vector.tensor_tensor(
                fsv,
                st[:, s:s + NDVE, 0:256].rearrange("p c (h f) -> p c h f",
                                                   h=4),
                wf[:, q:q + NDVE, :].unsqueeze(3).broadcast_to(
                    [128, NDVE, 4, 64]),
                AL.mult)
            for ci in range(NDVE, CM):
                for h in range(4):
                    nc.scalar.activation(fs[:, ci, h * 64:(h + 1) * 64],
                                         st[:, s + ci, h * 64:(h + 1) * 64],
                                         AF.Copy, scale=wf[:, q + ci, h:h + 1])

            agg = pagg.tile([128, 260], F32, tag="agg")
            for ci in range(CM):
                nc.tensor.matmul(agg[:], mkt[:, ci, :], fs[:, ci, 0:260],
                                 start=(ci == 0), stop=(ci == CM - 1))
            return (agg,)

        def back(bi, agg):
            recip = epool.tile([128, 4], F32, tag="recip")
            nc.vector.reciprocal(recip[:], agg[:, 256:260])
            if bi % SB == 0:
                state["xo"] = opool.tile([128, SB, 256], BF16, tag="xo", name="xo")
            xo = state["xo"][:, bi % SB, :]
            for h in range(2):
                sl = slice(h * 64, (h + 1) * 64)
                nc.vector.scalar_tensor_tensor(
                    xo[:, sl], agg[:, sl], recip[:, h:h + 1], btt[:, sl],
                    AL.mult, AL.add)
            for h in range(2, 4):
                sl = slice(h * 64, (h + 1) * 64)
                nc.scalar.activation(xo[:, sl], agg[:, sl], AF.Copy,
                                     scale=recip[:, h:h + 1])
            nc.vector.tensor_add(xo[:, 128:256], xo[:, 128:256],
                                 btt[:, 128:256])
            if bi % SB == SB - 1 or bi == NBLK - 1:
                b0 = bi - bi % SB
                nb = bi % SB + 1
                nc.gpsimd.dma_start(
                    out.rearrange("(b p) c -> p b c", p=128)[:, b0:b0 + nb, :],
                    state["xo"][:, 0:nb, :])

        prev = front(0)
        for bi in range(1, NBLK):
            cur = front(bi)
            back(bi - 1, *prev)
            prev = cur
        back(NBLK - 1, *prev)
    nc.compile()
    return nc


def _build_proj3():
    """h3scT = [W3|ws3|wd3].T @ elu(x3raw).T: in x3T [256, SHARD] bf16;
    out [4, SHARD] f32 (h3 2 | as3 | ad3), W stationary."""
    GB = 7                                   # 98 = 14 x 7
    ngrp = NBLK // GB
    nc = bacc.Bacc("TRN2", target_bir_lowering=False, debug=False,
                   num_devices=NCORE)
    xT = nc.dram_tensor("xT", [256, SHARD], BF16, kind="ExternalInput").ap()
    W = nc.dram_tensor("W", [256, 4], BF16, kind="ExternalInput").ap()
    out = nc.dram_tensor("out", [4, SHARD], F32, kind="ExternalOutput").ap()

    with tile.TileContext(nc) as tc, ExitStack() as ctx:
        cpool = ctx.enter_context(tc.tile_pool(name="c", bufs=1))
        pool = ctx.enter_context(tc.tile_pool(name="p", bufs=3))
        epool = ctx.enter_context(tc.tile_pool(name="e", bufs=3))
        psum = ctx.enter_context(tc.tile_pool(name="ps", bufs=2, space="PSUM"))
        wt = cpool.tile([128, 2, 4], BF16)
        for k in range(2):
            nc.sync.dma_start(wt[:, k, :], W[128 * k:128 * (k + 1), :])
        xTr = xT.rearrange("(k p) n -> p k n", p=128)
        for g in range(ngrp):
            xt = pool.tile([128, 2, GB, 128], BF16, tag="xt")
            nc.sync.dma_start(xt[:],
                              xTr[:, :, g * GB * 128:(g + 1) * GB * 128]
                              .rearrange("p k (b n) -> p k b n", n=128))
            mt = pool.tile([128, 2, GB, 128], BF16, tag="mt")
            nc.vector.tensor_scalar_min(mt[:], xt[:], 0.0)
            nc.scalar.activation(mt[:], mt[:], AF.Exp)
            nc.vector.scalar_tensor_tensor(xt[:], mt[:], 1.0, xt[:],
                                           AL.subtract, AL.max)
            ps = psum.tile([4, GB * 128], F32, tag="ps")
            for j in range(GB):
                for k in range(2):
                    nc.tensor.matmul(ps[:, j * 128:(j + 1) * 128],
                                     wt[:, k, :], xt[:, k, j, :],
                                     start=(k == 0), stop=(k == 1))
            ot = epool.tile([4, GB * 128], F32, tag="ot")
            nc.vector.tensor_copy(ot[:], ps[:])
            nc.gpsimd.dma_start(out[:, g * GB * 128:(g + 1) * GB * 128],
                                ot[:])
    nc.compile()
    return nc


def _build_agg3(cmax):
    """agg3: out[dst, 2] = (sum alpha h3_src) + b3, heads=1."""
    CM = cmax
    SB = 7                                    # 98 = 14 x 7
    W = 132                                   # fe 2 | mask 128 | pad 2
    nc = bacc.Bacc("TRN2", target_bir_lowering=False, debug=False,
                   num_devices=NCORE)
    fm = nc.dram_tensor("fm", [128, NBLK * CM * W], BF16,
                        kind="ExternalInput").ap()
    we = nc.dram_tensor("we", [128, NBLK * CM * 2], F32,
                        kind="ExternalInput").ap()
    bt3 = nc.dram_tensor("bt3", [128, 2], F32, kind="ExternalInput").ap()
    out = nc.dram_tensor("out", [SHARD, 2], F32, kind="ExternalOutput").ap()

    with tile.TileContext(nc) as tc, ExitStack() as ctx:
        cpool = ctx.enter_context(tc.tile_pool(name="c", bufs=1))
        spool = ctx.enter_context(tc.tile_pool(name="s", bufs=3))
        wpool = ctx.enter_context(tc.tile_pool(name="w", bufs=3))
        epool = ctx.enter_context(tc.tile_pool(name="e", bufs=3))
        pagg = ctx.enter_context(tc.tile_pool(name="pa", bufs=2, space="PSUM"))

        btt = cpool.tile([128, 2], F32)
        nc.sync.dma_start(btt[:], bt3[:])

        state = {}

        def front(sb):
            b0 = sb * SB
            c0 = b0 * CM
            nch = SB * CM
            st = spool.tile([128, SB * CM, W], BF16, tag="st")
            nc.sync.dma_start(st[:], fm[:, c0 * W:(c0 + nch) * W])
            wet = spool.tile([128, SB * CM, 2], F32, tag="wet")
            nc.sync.dma_start(wet[:], we[:, c0 * 2:(c0 + nch) * 2])

            zt = wpool.tile([128, SB * CM, 1], F32, tag="zt")
            nc.vector.tensor_add(zt[:], wet[:, :, 0:1], wet[:, :, 1:2])
            nc.vector.scalar_tensor_tensor(zt[:], zt[:], NEG, zt[:],
                                           AL.mult, AL.max)
            wf = wpool.tile([128, SB * CM, 1], F32, tag="wf")
            nc.scalar.activation(wf[:], zt[:], AF.Exp)
            rhs = wpool.tile([128, SB * CM, 4], BF16, tag="rhs")
            nc.vector.tensor_tensor(
                rhs[:, :, 0:2], st[:, :, 0:2],
                wf[:].broadcast_to([128, nch, 2]), AL.mult)
            nc.vector.tensor_copy(rhs[:, :, 2:3], wf[:])

            agg = pagg.tile([128, SB, 3], F32, tag="agg")
            for b in range(SB):
                for ci in range(CM):
                    cc = b * CM + ci
                    nc.tensor.matmul(agg[:, b, :], st[:, cc, 2:130],
                                     rhs[:, cc, 0:3],
                                     start=(ci == 0), stop=(ci == CM - 1),
                                     skip_group_check=(b > 0))
            return agg

        def back(sb, agg):
            b0 = sb * SB
            xs = epool.tile([128, SB, 2], F32, tag="xs")
            recip = epool.tile([128, SB, 1], F32, tag="recip")
            nc.vector.reciprocal(recip[:], agg[:, :, 2:3])
            for b in range(SB):
                nc.vector.scalar_tensor_tensor(
                    xs[:, b, :], agg[:, b, 0:2], recip[:, b, :], btt[:],
                    AL.mult, AL.add)
            outr = out.rearrange("(b p) c -> p b c", p=128)
            nc.gpsimd.dma_start(outr[:, b0:b0 + SB, :], xs[:])

        nsb = NBLK // SB
        prev = front(0)
        for sb in range(1, nsb):
            cur = front(sb)
            back(sb - 1, prev)
            prev = cur
        back(nsb - 1, prev)
    nc.compile()
    return nc


# --------------------------------------------------------------------------
# orchestration
# --------------------------------------------------------------------------

def _get_program(key, builder):
    if key not in _program_cache:
        _program_cache[key] = builder()
    return _program_cache[key]


def _run(stage, nc, in_maps):
    if TRACE:
        _install_profile_shim()
    res = run_bass_kernel_spmd(nc, in_maps, core_ids=list(range(NCORE)),
                               trace=TRACE, trace_cores=list(range(NCORE)),
                               stitch_traces=False)
    if res.exec_time_ns is not None:
        _last_stage_times[stage] = res.exec_time_ns
    return res


def _fold_ws(W, a, heads, out_c):
    Wr = W.reshape(W.shape[0], heads, out_c)
    return np.einsum('fhc,hc->fh', Wr.astype(np.float64),
                     a.astype(np.float64)).astype(np.float32)


def _bcast(b, outc, dtype):
    return np.ascontiguousarray(
        np.broadcast_to(np.asarray(b, np.float32).reshape(1, outc),
                        (128, outc)).astype(dtype))


def _pad_tab(tab):
    """[N, C] node table -> [NPAD+1, C] with zero pad rows (row N..: zeros)."""
    out = np.zeros((NPAD + 1, tab.shape[1]), tab.dtype)
    out[:tab.shape[0]] = tab
    return out


def kernel(x, edge_index, W1, a_src1, a_dst1, b1, W2, a_src2, a_dst2, b2,
           W3, a_src3, a_dst3, b3):
    x = np.asarray(x, np.float32)
    W1 = np.asarray(W1, np.float32); W2 = np.asarray(W2, np.float32)
    W3 = np.asarray(W3, np.float32)

    cores, cmax, slot_of_node, node_of_slot = _plan(np.asarray(edge_index))

    ws1 = _fold_ws(W1, np.asarray(a_src1, np.float32), 4, 128)
    wd1 = _fold_ws(W1, np.asarray(a_dst1, np.float32), 4, 128)
    ws2 = _fold_ws(W2, np.asarray(a_src2, np.float32), 4, 64)
    wd2 = _fold_ws(W2, np.asarray(a_dst2, np.float32), 4, 64)
    ws3 = _fold_ws(W3, np.asarray(a_src3, np.float32), 1, 2)
    wd3 = _fold_ws(W3, np.asarray(a_dst3, np.float32), 1, 2)

    # node features in slot order (xs[slot] = x[node_of_slot[slot]])
    xs = np.zeros((NPAD, 128), np.float32)
    real = node_of_slot < N
    xs[real] = x[node_of_slot[real]]

    # stage 1: as1/ad1 for all slots (shard-local rows)
    s1 = _get_program("proj1", _build_proj1)
    wsd1 = np.concatenate([ws1, wd1], axis=1)
    r1 = _run("proj1", s1, [
        {"xT": np.ascontiguousarray(xs[c * SHARD:(c + 1) * SHARD].T),
         "W": wsd1} for c in range(NCORE)])
    asad1 = np.concatenate([r1.results[c]["out"].T for c in range(NCORE)])
    # as1 by node id: asad1 is slot-indexed
    as1_node = np.zeros((NPAD + 1, 4), np.float32)
    as1_node[node_of_slot] = asad1[:, 0:4]
    ad1_slot = asad1[:, 4:8]

    # stage 2: agg1 + W1 proj -> x2raw (slot-major [NPAD, 512] bf16)
    xtab = np.zeros((NPAD + 1, 128), BF)       # node-indexed feature table
    xtab[:N] = x.astype(BF)
    s2 = _get_program(("agg1", cmax), lambda: _build_agg1(cmax))
    in2 = [{"fm": _fm_stream(cores[c], xtab),
            "we": _we_stream(cores[c], as1_node, ad1_slot, c, cmax),
            "W1b": W1.astype(BF),
            "bt1": _bcast(b1, 512, BF)} for c in range(NCORE)]
    r2 = _run("agg1", s2, in2)
    x2 = np.concatenate([r2.results[c]["out"] for c in range(NCORE)])

    # stage 3: proj2 (elu + W2|ws2|wd2)
    s3 = _get_program("proj2", _build_proj2)
    W2s = np.concatenate([W2, ws2, wd2], axis=1).astype(BF)
    r3 = _run("proj2", s3, [
        {"xT": np.ascontiguousarray(x2[c * SHARD:(c + 1) * SHARD].T),
         "W": W2s} for c in range(NCORE)])
    h2 = np.concatenate([r3.results[c]["oh"] for c in range(NCORE)])
    sc2 = np.concatenate([r3.results[c]["osc"] for c in range(NCORE)])
    as2_node = np.zeros((NPAD + 1, 4), np.float32)
    as2_node[node_of_slot] = sc2[:, 0:4]
    ad2_slot = sc2[:, 4:8]

    # stage 4: agg2 -> x3raw
    h2tab = np.zeros((NPAD + 1, 256), BF)
    h2tab[node_of_slot] = h2
    s4 = _get_program(("agg2", cmax), lambda: _build_agg2(cmax))
    in4 = [{"fm": _fm_stream(cores[c], h2tab),
            "we": _we_stream(cores[c], as2_node, ad2_slot, c, cmax),
            "bt2": _bcast(b2, 256, BF)} for c in range(NCORE)]
    r4 = _run("agg2", s4, in4)
    x3 = np.concatenate([r4.results[c]["out"] for c in range(NCORE)])

    # stage 5: proj3 (elu + W3|ws3|wd3), W stationary, transposed out
    s5 = _get_program("proj3", _build_proj3)
    W3s = np.concatenate([W3, ws3, wd3], axis=1).astype(BF)
    r5 = _run("proj3", s5, [
        {"xT": np.ascontiguousarray(x3[c * SHARD:(c + 1) * SHARD].T),
         "W": W3s} for c in range(NCORE)])
    h3sc = np.concatenate([r5.results[c]["out"].T for c in range(NCORE)])
    h3_slot, as3_slot, ad3_slot = (h3sc[:, 0:2], h3sc[:, 2:3], h3sc[:, 3:4])
    h3tab = np.zeros((NPAD + 1, 2), BF)
    h3tab[node_of_slot] = h3_slot
    as3_node = np.zeros((NPAD + 1, 1), np.float32)
    as3_node[node_of_slot] = as3_slot

    # stage 6: agg3 -> out
    s6 = _get_program(("agg3", cmax), lambda: _build_agg3(cmax))
    in6 = [{"fm": _fm_stream(cores[c], h3tab, pad=2),
            "we": _we_stream(cores[c], as3_node, ad3_slot, c, cmax),
            "bt3": _bcast(b3, 2, np.float32)} for c in range(NCORE)]
    r6 = _run("agg3", s6, in6)
    outp = np.concatenate([r6.results[c]["out"] for c in range(NCORE)])
    return np.ascontiguousarray(outp[slot_of_node[:N]]).astype(np.float32)


# revision 28
# speedup vs baseline: 4.4606x; 1.0796x over previous
"""3-layer GAT (PyG GATConv-style) on 8 Trainium2 NeuronCores — v2.

Strategy (dst-node sharding, all fp math on device):
  - Nodes are permuted into 100352 slots = 8 shards x 98 blocks x 128 via an
    LPT bin-pack on in-degree so every block has ~equal incoming-edge count
    (cmax = ceil(max_block_edges/128) drops 6 -> 5).
  - Edges (incl. self-loops) are partitioned by dst block; per (core, block)
    the edge list is padded to cmax 128-edge chunks; all 8 cores run one SPMD
    program per stage.
  - Segment softmax + weighted segment-sum run on TensorE via HOST-BUILT
    one-hot dst masks (plain DMA; no device gather).  The softmax max-shift
    is dropped (shift invariance; logits are O(10), safe in fp32 exp range);
    the denominator comes from a mask.T @ w matmul and is applied at PSUM
    evacuation.  Self-loops guarantee den > 0, so no epsilon term.
  - 6 SPMD stages: proj1 (as/ad), agg1(+W1 proj), proj2 (elu+W2|as2|ad2),
    agg2, proj3 (elu+W3|as3|ad3), agg3.  Host glue does layout only
    (permutation, row gathers by edge index, transpose, pad).
"""
import sys

sys.path.insert(0, "/opt/trn_rl_repo")

import heapq
import numpy as np
import ml_dtypes
from contextlib import ExitStack

import concourse.bass as bass
import concourse.bacc as bacc
import concourse.tile as tile
import concourse.mybir as mybir
from concourse.bass_utils import run_bass_kernel_spmd

F32 = mybir.dt.float32
F32R = mybir.dt.float32r
BF16 = mybir.dt.bfloat16
AL = mybir.AluOpType
AF = mybir.ActivationFunctionType
BF = ml_dtypes.bfloat16

N = 100000
NPAD = 100352            # 8 * 98 * 128
NCORE = 8
SHARD = NPAD // NCORE    # 12544
NBLK = SHARD // 128      # 98
NBLK_ALL = NPAD // 128   # 784
NEG = 0.2

_program_cache = {}
_last_stage_times = {}   # stage -> exec_time_ns (filled when trace=True)
TRACE = False
_shim_done = [False]


def _install_profile_shim():
    """The agent image's antenv lacks axon_hooks; recreate the tiny shim so
    run_bass_kernel_spmd(trace=True) can drive NTFF profiling via the axon
    plugin's C ABI, and stub the S3 artifact upload (no creds here)."""
    if _shim_done[0]:
        return
    import types
    mod = types.ModuleType("antenv.axon_hooks")
    holder = [None]
    mod.set_axon_ntff_profile_hook = lambda h: holder.__setitem__(0, h)
    mod.get_axon_ntff_profile_hook = lambda: holder[0]
    sys.modules["antenv.axon_hooks"] = mod
    from trn_agent_boot.trn_boot import _ntff_profile_via_ctypes
    holder[0] = _ntff_profile_via_ctypes('/opt/axon/libaxon_pjrt.so')
    import concourse.bass_utils as bu
    bu.upload_artifacts = lambda tmpdir: "local://" + str(tmpdir)
    _shim_done[0] = True


# --------------------------------------------------------------------------
# host-side preprocessing (indices / layout only)
# --------------------------------------------------------------------------

def _balance_slots(deg):
    """LPT bin-pack: assign nodes (by desc in-degree) to 784 blocks of 128
    slots so block edge counts are ~equal.  Returns slot_of_node[NPAD]."""
    order = np.argsort(-deg, kind="stable")
    heap = [(0, 0, b) for b in range(NBLK_ALL)]
    heapq.heapify(heap)
    slot_of_node = np.empty(NPAD, np.int64)
    fill = np.zeros(NBLK_ALL, np.int32)
    # process in runs of equal degree: round-robin via heap
    for n in order:
        s, c, b = heapq.heappop(heap)
        slot_of_node[n] = b * 128 + fill[b]
        fill[b] += 1
        c += 1
        if c < 128:
            heapq.heappush(heap, (s + int(deg[n]), c, b))
    return slot_of_node


def _plan(edge_index):
    src = np.concatenate([np.asarray(edge_index[0], np.int64),
                          np.arange(N, dtype=np.int64)])
    dst = np.concatenate([np.asarray(edge_index[1], np.int64),
                          np.arange(N, dtype=np.int64)])
    deg = np.bincount(dst, minlength=NPAD)  # padded "nodes" N..NPAD-1: deg 0
    slot_of_node = _balance_slots(deg)
    node_of_slot = np.empty(NPAD, np.int64)
    node_of_slot[slot_of_node] = np.arange(NPAD)

    dslot = slot_of_node[dst]
    blk = dslot // 128
    order = np.argsort(blk * 256 + (dslot % 128) // 64, kind="stable")
    src, dslot, blk = src[order], dslot[order], blk[order]
    bc = np.bincount(blk, minlength=NBLK_ALL)
    cmax = int(np.ceil(bc.max() / 128))
    starts = np.zeros(NBLK_ALL + 1, np.int64)
    np.cumsum(bc, out=starts[1:])
    L = cmax * 128

    # token slot for each edge: block-local position + block base
    within = np.arange(len(src)) - starts[blk]
    tok = blk * L + within                      # global padded token index
    T_all = NBLK_ALL * L
    tok_src = np.full(T_all, N, np.int64)       # N -> zero row in tables
    tok_dstl = np.full(T_all, 0, np.int64)
    tok_valid = np.zeros(T_all, bool)
    tok_src[tok] = src
    tok_dstl[tok] = dslot % 128
    tok_valid[tok] = True

    Tc = NBLK * L                               # tokens per core
    cores = []
    for c in range(NCORE):
        sl = slice(c * Tc, (c + 1) * Tc)
        cores.append(dict(src=tok_src[sl], dstl=tok_dstl[sl],
                          valid=tok_valid[sl]))
    return cores, cmax, slot_of_node, node_of_slot


def _chunkmaj(rows):
    """[T, C] row-major -> [128, T/128 * C] token-partition-major."""
    Tn, C = rows.shape
    ch = rows.reshape(Tn // 128, 128, C).transpose(1, 0, 2).reshape(128, -1)
    return np.ascontiguousarray(ch)


def _stream(core_plan, table, dtype):
    """Host row-gather by token src id -> [128, T/128 * C]."""
    Tn = core_plan["src"].shape[0]
    C = table.shape[1]
    rows = np.zeros((Tn, C), dtype)
    v = core_plan["valid"]
    rows[v] = table[core_plan["src"][v]].astype(dtype)
    return _chunkmaj(rows)


def _fm_stream(core_plan, table, pad=0):
    """Host [gathered-features | one-hot mask | pad] -> [128, T/128*W] bf16."""
    Tn = core_plan["src"].shape[0]
    C = table.shape[1]
    rows = np.zeros((Tn, C + 128 + pad), BF)
    v = np.nonzero(core_plan["valid"])[0]
    rows[v, 0:C] = table[core_plan["src"][v]]
    rows[v, C + core_plan["dstl"][v]] = 1
    return _chunkmaj(rows)


def _we_stream(core_plan, src_tab, dst_tab, core_id, cmax):
    """[as(src) | ad(dst)] per token -> [128, T/128 * 2H] f32.
    src_tab/dst_tab are [NPAD(+1), H] node-indexed (row N.. = zeros)."""
    Tn = core_plan["src"].shape[0]
    H = src_tab.shape[1]
    rows = np.zeros((Tn, 2 * H), np.float32)
    v = core_plan["valid"]
    rows[v, :H] = src_tab[core_plan["src"][v]]
    L = cmax * 128
    blkl = np.arange(Tn) // L
    dst_slot = (core_id * NBLK + blkl) * 128 + core_plan["dstl"]
    rows[v, H:] = dst_tab[dst_slot[v]]          # dst_tab slot-indexed
    return _chunkmaj(rows)


# --------------------------------------------------------------------------
# stage program builders
# --------------------------------------------------------------------------

def _build_proj1():
    """asadT[8, SHARD] = ([ws1|wd1].T @ xT) with W stationary."""
    GB = 7                                   # blocks per group (98 = 14 x 7)
    ngrp = NBLK // GB
    nc = bacc.Bacc("TRN2", target_bir_lowering=False, debug=False,
                   num_devices=NCORE)
    xT = nc.dram_tensor("xT", [128, SHARD], F32, kind="ExternalInput").ap()
    W = nc.dram_tensor("W", [128, 8], F32, kind="ExternalInput").ap()
    out = nc.dram_tensor("out", [8, SHARD], F32, kind="ExternalOutput").ap()

    with tile.TileContext(nc) as tc, ExitStack() as ctx:
        cpool = ctx.enter_context(tc.tile_pool(name="c", bufs=1))
        pool = ctx.enter_context(tc.tile_pool(name="p", bufs=4))
        epool = ctx.enter_context(tc.tile_pool(name="e", bufs=4))
        psum = ctx.enter_context(tc.tile_pool(name="ps", bufs=3, space="PSUM"))
        wt = cpool.tile([128, 8], F32)
        nc.sync.dma_start(wt[:], W[:])
        xTr = xT.rearrange("p (b n) -> p b n", n=128)
        for g in range(ngrp):
            xt = pool.tile([128, GB, 128], F32, tag="xt")
            nc.sync.dma_start(xt[:], xTr[:, g * GB:(g + 1) * GB, :])
            ps = psum.tile([8, GB * 128], F32, tag="ps")
            for j in range(GB):
                nc.tensor.matmul(ps[:, j * 128:(j + 1) * 128],
                                 wt[:], xt[:, j, :],
                                 start=True, stop=True)
            ot = epool.tile([8, GB * 128], F32, tag="ot")
            nc.vector.tensor_copy(ot[:], ps[:])
            nc.gpsimd.dma_start(out[:, g * GB * 128:(g + 1) * GB * 128],
                                ot[:])
    nc.compile()
    return nc


def _build_agg1(cmax):
    """agg1 + W1 proj: x2raw[dst, 512] = (sum alpha x_src) @ W1 + b1
    (no elu here; proj2 applies it).  Software-pipelined: heavy PSUM-side
    work of block i is emitted after the stream work of block i+1."""
    CM = cmax
    WEB = 4                                   # blocks per we-DMA batch
    SB = 2                                    # blocks per stream-DMA batch
    NACT = max(1, CM // 4)                    # wm chunks built on ScalarE
    NDVE = CM - NACT                          # wm chunks built on VectorE
    W = 256                                   # stream width: fe 128 | mask 128
    nc = bacc.Bacc("TRN2", target_bir_lowering=False, debug=False,
                   num_devices=NCORE)
    fm = nc.dram_tensor("fm", [128, NBLK * CM * W], BF16,
                        kind="ExternalInput").ap()
    we = nc.dram_tensor("we", [128, NBLK * CM * 8], F32,
                        kind="ExternalInput").ap()
    W1b = nc.dram_tensor("W1b", [128, 512], BF16, kind="ExternalInput").ap()
    bt1 = nc.dram_tensor("bt1", [128, 512], BF16, kind="ExternalInput").ap()
    out = nc.dram_tensor("out", [SHARD, 512], BF16, kind="ExternalOutput").ap()

    with tile.TileContext(nc) as tc, ExitStack() as ctx:
        cpool = ctx.enter_context(tc.tile_pool(name="c", bufs=1))
        spool = ctx.enter_context(tc.tile_pool(name="s", bufs=3))
        vpool = ctx.enter_context(tc.tile_pool(name="v", bufs=2))
        wpool = ctx.enter_context(tc.tile_pool(name="w", bufs=3))
        epool = ctx.enter_context(tc.tile_pool(name="e", bufs=2))
        opool = ctx.enter_context(tc.tile_pool(name="o", bufs=2))
        pagg = ctx.enter_context(tc.tile_pool(name="pa", bufs=2, space="PSUM"))
        pden = ctx.enter_context(tc.tile_pool(name="pd", bufs=2, space="PSUM"))
        po1 = ctx.enter_context(tc.tile_pool(name="po", bufs=2, space="PSUM"))

        w1t = cpool.tile([128, 512], BF16)
        nc.sync.dma_start(w1t[:], W1b[:])
        btt = cpool.tile([128, 512], BF16)
        nc.sync.dma_start(btt[:], bt1[:])

        state = {}

        def front(bi):
            c0 = bi * CM
            if bi % WEB == 0:
                nw = min(WEB, NBLK - bi) * CM
                wet = vpool.tile([128, WEB * CM, 8], F32, tag="wet")
                nc.sync.dma_start(wet[:, 0:nw, :], we[:, c0 * 8:(c0 + nw) * 8])
                zt = vpool.tile([128, WEB * CM, 4], F32, tag="zt")
                nc.vector.tensor_add(zt[:, 0:nw, :], wet[:, 0:nw, 0:4],
                                     wet[:, 0:nw, 4:8])
                nc.vector.scalar_tensor_tensor(zt[:, 0:nw, :], zt[:, 0:nw, :],
                                               NEG, zt[:, 0:nw, :],
                                               AL.mult, AL.max)
                wf = vpool.tile([128, WEB * CM, 4], F32, tag="wf")
                nc.scalar.activation(wf[:, 0:nw, :], zt[:, 0:nw, :], AF.Exp)
                wb = vpool.tile([128, WEB * CM, 4], BF16, tag="wb")
                nc.vector.tensor_copy(wb[:, 0:nw, :], wf[:, 0:nw, :])
                state["wf"], state["wb"] = wf, wb
            if bi % SB == 0:
                ns = min(SB, NBLK - bi) * CM
                st = spool.tile([128, SB * CM, W], BF16, tag="st")
                nc.sync.dma_start(st[:, 0:ns, :], fm[:, c0 * W:(c0 + ns) * W])
                state["st"] = st
            wf, wb = state["wf"], state["wb"]
            st = state["st"]
            q = (bi % WEB) * CM
            s = (bi % SB) * CM
            fet = st[:, s:s + CM, 0:128]
            mkt = st[:, s:s + CM, 128:256]

            wm = wpool.tile([128, CM, 4, 128], BF16, tag="wm")
            nc.vector.tensor_tensor(
                wm[:, 0:NDVE, :, :],
                st[:, s:s + NDVE, 128:256].unsqueeze(2).broadcast_to(
                    [128, NDVE, 4, 128]),
                wf[:, q:q + NDVE, :].unsqueeze(3).broadcast_to(
                    [128, NDVE, 4, 128]),
                AL.mult)
            for ci in range(NDVE, CM):
                for h in range(4):
                    nc.scalar.activation(wm[:, ci, h, :],
                                         st[:, s + ci, 128:256],
                                         AF.Copy, scale=wf[:, q + ci, h:h + 1])

            aggT = pagg.tile([128, 512], F32, tag="aggT")
            den = pden.tile([128, 4], F32, tag="den")
            for ci in range(CM):
                nc.tensor.matmul(
                    aggT[:], fet[:, ci, :],
                    wm[:, ci, :, :].rearrange("p h d -> p (h d)"),
                    start=(ci == 0), stop=(ci == CM - 1))
            for ci in range(CM):
                nc.tensor.matmul(den[:], mkt[:, ci, :], wb[:, q + ci, :],
                                 start=(ci == 0), stop=(ci == CM - 1),
                                 skip_group_check=True)
            return aggT, den

        def back(bi, aggT, den):
            recip = epool.tile([128, 4], F32, tag="recip")
            nc.vector.reciprocal(recip[:], den[:])
            aggTs = epool.tile([128, 512], BF16, tag="aggTs")
            nc.vector.tensor_copy(aggTs[:, 0:256], aggT[:, 0:256])
            nc.scalar.activation(aggTs[:, 256:512], aggT[:, 256:512], AF.Copy)
            o1 = po1.tile([128, 512], F32, tag="o1")
            for h in range(4):
                sl = slice(h * 128, (h + 1) * 128)
                nc.tensor.matmul(o1[:, sl], aggTs[:, sl], w1t[:, sl],
                                 start=True, stop=True)
            if bi % SB == 0:
                state["xo"] = opool.tile([128, SB, 512], BF16, tag="xo", name="xo")
            xo = state["xo"][:, bi % SB, :]
            for h in range(2):
                sl = slice(h * 128, (h + 1) * 128)
                nc.vector.scalar_tensor_tensor(
                    xo[:, sl], o1[:, sl], recip[:, h:h + 1], btt[:, sl],
                    AL.mult, AL.add)
            for h in range(2, 4):
                sl = slice(h * 128, (h + 1) * 128)
                nc.scalar.activation(xo[:, sl], o1[:, sl], AF.Copy,
                                     scale=recip[:, h:h + 1])
            nc.vector.tensor_add(xo[:, 256:512], xo[:, 256:512],
                                 btt[:, 256:512])
            if bi % SB == SB - 1 or bi == NBLK - 1:
                b0 = bi - bi % SB
                nb = bi % SB + 1
                nc.gpsimd.dma_start(
                    out.rearrange("(b p) c -> p b c", p=128)[:, b0:b0 + nb, :],
                    state["xo"][:, 0:nb, :])

        prev = front(0)
        for bi in range(1, NBLK):
            cur = front(bi)
            back(bi - 1, *prev)
            prev = cur
        back(NBLK - 1, *prev)
    nc.compile()
    return nc


def _build_proj2():
    """h2sc = elu(x2raw).T-proj: in x2T [512, SHARD] bf16;
    out h2 [SHARD, 256] bf16 + sc [SHARD, 8] f32 (as2|ad2)."""
    nc = bacc.Bacc("TRN2", target_bir_lowering=False, debug=False,
                   num_devices=NCORE)
    xT = nc.dram_tensor("xT", [512, SHARD], BF16, kind="ExternalInput").ap()
    W = nc.dram_tensor("W", [512, 264], BF16, kind="ExternalInput").ap()
    oh = nc.dram_tensor("oh", [SHARD, 256], BF16, kind="ExternalOutput").ap()
    osc = nc.dram_tensor("osc", [SHARD, 8], F32, kind="ExternalOutput").ap()

    with tile.TileContext(nc) as tc, ExitStack() as ctx:
        cpool = ctx.enter_context(tc.tile_pool(name="c", bufs=1))
        pool = ctx.enter_context(tc.tile_pool(name="p", bufs=3))
        epool = ctx.enter_context(tc.tile_pool(name="e", bufs=2))
        psum = ctx.enter_context(tc.tile_pool(name="ps", bufs=3, space="PSUM"))
        SB = 2
        GB = 8                                   # blocks of sc staged per DMA
        wt = cpool.tile([128, 4, 264], BF16)
        for k in range(4):
            nc.sync.dma_start(wt[:, k, :], W[128 * k:128 * (k + 1), :])
        xTr = xT.rearrange("(k p) n -> p k n", p=128)
        state = {}

        def front(bi):
            if bi % SB == 0:
                n = min(SB, NBLK - bi) * 128
                xt = pool.tile([128, 4, SB * 128], BF16, tag="xt")
                nc.sync.dma_start(xt[:, :, 0:n],
                                  xTr[:, :, 128 * bi:128 * bi + n])
                # elu in place: xe = max(exp(min(x,0))-1, x)
                mt = pool.tile([128, 4, SB * 128], BF16, tag="mt")
                nc.vector.tensor_scalar_min(mt[:, :, 0:n], xt[:, :, 0:n], 0.0)
                nc.scalar.activation(mt[:, :, 0:n], mt[:, :, 0:n], AF.Exp)
                nc.vector.scalar_tensor_tensor(xt[:, :, 0:n], mt[:, :, 0:n],
                                               1.0, xt[:, :, 0:n],
                                               AL.subtract, AL.max)
                state["xt"] = xt
            xt = state["xt"]
            s = (bi % SB) * 128
            ps = psum.tile([128, 264], F32, tag="ps")
            for k in range(4):
                nc.tensor.matmul(ps[:], xt[:, k, s:s + 128], wt[:, k, :],
                                 start=(k == 0), stop=(k == 3))
            return (ps,)

        def back(bi, ps):
            if bi % SB == 0:
                state["ht"] = epool.tile([128, SB, 256], BF16, tag="ht", name="ht")
            ht = state["ht"][:, bi % SB, :]
            nc.vector.tensor_copy(ht[:, 0:128], ps[:, 0:128])
            nc.scalar.activation(ht[:, 128:256], ps[:, 128:256], AF.Copy)
            if bi % SB == SB - 1 or bi == NBLK - 1:
                b0 = bi - bi % SB
                nb = bi % SB + 1
                nc.gpsimd.dma_start(
                    oh.rearrange("(b p) c -> p b c", p=128)[:, b0:b0 + nb, :],
                    state["ht"][:, 0:nb, :])
            g, j = bi // GB, bi % GB
            if j == 0:
                state["sct"] = epool.tile([128, GB, 8], F32, tag="sct", name="sct")
            nc.vector.tensor_copy(state["sct"][:, j, :], ps[:, 256:264])
            if j == GB - 1 or bi == NBLK - 1:
                nb = j + 1
                oscr = osc.rearrange("(b p) c -> p b c", p=128)
                nc.gpsimd.dma_start(oscr[:, g * GB:g * GB + nb, :],
                                    state["sct"][:, 0:nb, :])

        prev = front(0)
        for bi in range(1, NBLK):
            cur = front(bi)
            back(bi - 1, *prev)
            prev = cur
        back(NBLK - 1, *prev)
    nc.compile()
    return nc


def _build_agg2(cmax):
    """agg2: x3raw[dst, 256] = (sum alpha h2_src) + b2 (no elu; proj3)."""
    CM = cmax
    W = 384                                   # fe 256 | mask 128
    nc = bacc.Bacc("TRN2", target_bir_lowering=False, debug=False,
                   num_devices=NCORE)
    fm = nc.dram_tensor("fm", [128, NBLK * CM * W], BF16,
                        kind="ExternalInput").ap()
    we = nc.dram_tensor("we", [128, NBLK * CM * 8], F32,
                        kind="ExternalInput").ap()
    bt2 = nc.dram_tensor("bt2", [128, 256], BF16, kind="ExternalInput").ap()
    out = nc.dram_tensor("out", [SHARD, 256], BF16, kind="ExternalOutput").ap()

    with tile.TileContext(nc) as tc, ExitStack() as ctx:
        cpool = ctx.enter_context(tc.tile_pool(name="c", bufs=1))
        spool = ctx.enter_context(tc.tile_pool(name="s", bufs=3))
        vpool = ctx.enter_context(tc.tile_pool(name="v", bufs=2))
        wpool = ctx.enter_context(tc.tile_pool(name="w", bufs=3))
        epool = ctx.enter_context(tc.tile_pool(name="e", bufs=2))
        opool = ctx.enter_context(tc.tile_pool(name="o", bufs=2))
        pagg = ctx.enter_context(tc.tile_pool(name="pa", bufs=3, space="PSUM"))

        WEB = 4
        SB = 2
        NACT = 1                              # fs chunks built on ScalarE
        NDVE = CM - NACT
        btt = cpool.tile([128, 256], BF16)
        nc.sync.dma_start(btt[:], bt2[:])

        state = {}

        def front(bi):
            c0 = bi * CM
            if bi % WEB == 0:
                nw = min(WEB, NBLK - bi) * CM
                wet = vpool.tile([128, WEB * CM, 8], F32, tag="wet")
                nc.sync.dma_start(wet[:, 0:nw, :], we[:, c0 * 8:(c0 + nw) * 8])
                zt = vpool.tile([128, WEB * CM, 4], F32, tag="zt")
                nc.vector.tensor_add(zt[:, 0:nw, :], wet[:, 0:nw, 0:4],
                                     wet[:, 0:nw, 4:8])
                nc.vector.scalar_tensor_tensor(zt[:, 0:nw, :], zt[:, 0:nw, :],
                                               NEG, zt[:, 0:nw, :],
                                               AL.mult, AL.max)
                wf = vpool.tile([128, WEB * CM, 4], F32, tag="wf")
                nc.scalar.activation(wf[:, 0:nw, :], zt[:, 0:nw, :], AF.Exp)
                state["wf"] = wf
            if bi % SB == 0:
                ns = min(SB, NBLK - bi) * CM
                st = spool.tile([128, SB * CM, W], BF16, tag="st")
                nc.sync.dma_start(st[:, 0:ns, :], fm[:, c0 * W:(c0 + ns) * W])
                state["st"] = st
            wf = state["wf"]
            st = state["st"]
            q = (bi % WEB) * CM
            s = (bi % SB) * CM
            mkt = st[:, s:s + CM, 256:384]
            # fs laid out [128, CM, 264]: cols 0:256 scaled features,
            # cols 256:260 = w (bf16) so one rhs serves agg+den.
            fs = wpool.tile([128, CM, 264], BF16, tag="fs")
            nc.vector.tensor_copy(fs[:, :, 256:260], wf[:, q:q + CM, :])
            fsv = fs[:, 0:NDVE, 0:256].rearrange("p c (h f) -> p c h f", h=4)
            nc.vector.tensor_tensor(
                fsv,
                st[:, s:s + NDVE, 0:256].rearrange("p c (h f) -> p c h f",
                                                   h=4),
                wf[:, q:q + NDVE, :].unsqueeze(3).broadcast_to(
                    [128, NDVE, 4, 64]),
                AL.mult)
            for ci in range(NDVE, CM):
                for h in range(4):
                    nc.scalar.activation(fs[:, ci, h * 64:(h + 1) * 64],
                                         st[:, s + ci, h * 64:(h + 1) * 64],
                                         AF.Copy, scale=wf[:, q + ci, h:h + 1])

            agg = pagg.tile([128, 260], F32, tag="agg")
            for ci in range(CM):
                nc.tensor.matmul(agg[:], mkt[:, ci, :], fs[:, ci, 0:260],
                                 start=(ci == 0), stop=(ci == CM - 1))
            return (agg,)

        def back(bi, agg):
            recip = epool.tile([128, 4], F32, tag="recip")
            nc.vector.reciprocal(recip[:], agg[:, 256:260])
            if bi % SB == 0:
                state["xo"] = opool.tile([128, SB, 256], BF16, tag="xo", name="xo")
            xo = state["xo"][:, bi % SB, :]
            for h in range(2):
                sl = slice(h * 64, (h + 1) * 64)
                nc.vector.scalar_tensor_tensor(
                    xo[:, sl], agg[:, sl], recip[:, h:h + 1], btt[:, sl],
                    AL.mult, AL.add)
            for h in range(2, 4):
                sl = slice(h * 64, (h + 1) * 64)
                nc.scalar.activation(xo[:, sl], agg[:, sl], AF.Copy,
                                     scale=recip[:, h:h + 1])
            nc.vector.tensor_add(xo[:, 128:256], xo[:, 128:256],
                                 btt[:, 128:256])
            if bi % SB == SB - 1 or bi == NBLK - 1:
                b0 = bi - bi % SB
                nb = bi % SB + 1
                nc.gpsimd.dma_start(
                    out.rearrange("(b p) c -> p b c", p=128)[:, b0:b0 + nb, :],
                    state["xo"][:, 0:nb, :])

        prev = front(0)
        for bi in range(1, NBLK):
            cur = front(bi)
            back(bi - 1, *prev)
            prev = cur
        back(NBLK - 1, *prev)
    nc.compile()
    return nc


def _build_proj3():
    """h3scT = [W3|ws3|wd3].T @ elu(x3raw).T: in x3T [256, SHARD] bf16;
    out [4, SHARD] f32 (h3 2 | as3 | ad3), W stationary."""
    GB = 7                                   # 98 = 14 x 7
    ngrp = NBLK // GB
    nc = bacc.Bacc("TRN2", target_bir_lowering=False, debug=False,
                   num_devices=NCORE)
    xT = nc.dram_tensor("xT", [256, SHARD], BF16, kind="ExternalInput").ap()
    W = nc.dram_tensor("W", [256, 4], BF16, kind="ExternalInput").ap()
    out = nc.dram_tensor("out", [4, SHARD], F32, kind="ExternalOutput").ap()

    with tile.TileContext(nc) as tc, ExitStack() as ctx:
        cpool = ctx.enter_context(tc.tile_pool(name="c", bufs=1))
        pool = ctx.enter_context(tc.tile_pool(name="p", bufs=4))
        epool = ctx.enter_context(tc.tile_pool(name="e", bufs=4))
        psum = ctx.enter_context(tc.tile_pool(name="ps", bufs=3, space="PSUM"))
        wt = cpool.tile([128, 2, 4], BF16)
        for k in range(2):
            nc.sync.dma_start(wt[:, k, :], W[128 * k:128 * (k + 1), :])
        xTr = xT.rearrange("(k p) n -> p k n", p=128)
        for g in range(ngrp):
            xt = pool.tile([128, 2, GB, 128], BF16, tag="xt")
            nc.sync.dma_start(xt[:],
                              xTr[:, :, g * GB * 128:(g + 1) * GB * 128]
                              .rearrange("p k (b n) -> p k b n", n=128))
            mt = pool.tile([128, 2, GB, 128], BF16, tag="mt")
            nc.vector.tensor_scalar_min(mt[:], xt[:], 0.0)
            nc.scalar.activation(mt[:], mt[:], AF.Exp)
            nc.vector.scalar_tensor_tensor(xt[:], mt[:], 1.0, xt[:],
                                           AL.subtract, AL.max)
            ps = psum.tile([4, GB * 128], F32, tag="ps")
            for j in range(GB):
                for k in range(2):
                    nc.tensor.matmul(ps[:, j * 128:(j + 1) * 128],
                                     wt[:, k, :], xt[:, k, j, :],
                                     start=(k == 0), stop=(k == 1))
            ot = epool.tile([4, GB * 128], F32, tag="ot")
            nc.vector.tensor_copy(ot[:], ps[:])
            nc.gpsimd.dma_start(out[:, g * GB * 128:(g + 1) * GB * 128],
                                ot[:])
    nc.compile()
    return nc


def _build_agg3(cmax):
    """agg3: out[dst, 2] = (sum alpha h3_src) + b3, heads=1."""
    CM = cmax
    SB = 7                                    # 98 = 14 x 7
    W = 132                                   # fe 2 | mask 128 | pad 2
    nc = bacc.Bacc("TRN2", target_bir_lowering=False, debug=False,
                   num_devices=NCORE)
    fm = nc.dram_tensor("fm", [128, NBLK * CM * W], BF16,
                        kind="ExternalInput").ap()
    we = nc.dram_tensor("we", [128, NBLK * CM * 2], F32,
                        kind="ExternalInput").ap()
    bt3 = nc.dram_tensor("bt3", [128, 2], F32, kind="ExternalInput").ap()
    out = nc.dram_tensor("out", [SHARD, 2], F32, kind="ExternalOutput").ap()

    with tile.TileContext(nc) as tc, ExitStack() as ctx:
        cpool = ctx.enter_context(tc.tile_pool(name="c", bufs=1))
        spool = ctx.enter_context(tc.tile_pool(name="s", bufs=4))
        wpool = ctx.enter_context(tc.tile_pool(name="w", bufs=4))
        epool = ctx.enter_context(tc.tile_pool(name="e", bufs=4))
        pagg = ctx.enter_context(tc.tile_pool(name="pa", bufs=3, space="PSUM"))

        btt = cpool.tile([128, 2], F32)
        nc.sync.dma_start(btt[:], bt3[:])

        state = {}

        def front(sb):
            b0 = sb * SB
            c0 = b0 * CM
            nch = SB * CM
            st = spool.tile([128, SB * CM, W], BF16, tag="st")
            nc.sync.dma_start(st[:], fm[:, c0 * W:(c0 + nch) * W])
            wet = spool.tile([128, SB * CM, 2], F32, tag="wet")
            nc.scalar.dma_start(wet[:], we[:, c0 * 2:(c0 + nch) * 2])

            zt = wpool.tile([128, SB * CM, 1], F32, tag="zt")
            nc.vector.tensor_add(zt[:], wet[:, :, 0:1], wet[:, :, 1:2])
            nc.vector.scalar_tensor_tensor(zt[:], zt[:], NEG, zt[:],
                                           AL.mult, AL.max)
            wf = wpool.tile([128, SB * CM, 1], F32, tag="wf")
            nc.scalar.activation(wf[:], zt[:], AF.Exp)
            rhs = wpool.tile([128, SB * CM, 4], BF16, tag="rhs")
            nc.vector.tensor_tensor(
                rhs[:, :, 0:2], st[:, :, 0:2],
                wf[:].broadcast_to([128, nch, 2]), AL.mult)
            nc.vector.tensor_copy(rhs[:, :, 2:3], wf[:])

            agg = pagg.tile([128, SB, 3], F32, tag="agg")
            for b in range(SB):
                for ci in range(CM):
                    cc = b * CM + ci
                    nc.tensor.matmul(agg[:, b, :], st[:, cc, 2:130],
                                     rhs[:, cc, 0:3],
                                     start=(ci == 0), stop=(ci == CM - 1),
                                     skip_group_check=(b > 0))
            return agg

        def back(sb, agg):
            b0 = sb * SB
            xs = epool.tile([128, SB, 2], F32, tag="xs")
            recip = epool.tile([128, SB, 1], F32, tag="recip")
            nc.vector.reciprocal(recip[:], agg[:, :, 2:3])
            for b in range(SB):
                nc.vector.scalar_tensor_tensor(
                    xs[:, b, :], agg[:, b, 0:2], recip[:, b, :], btt[:],
                    AL.mult, AL.add)
            outr = out.rearrange("(b p) c -> p b c", p=128)
            nc.gpsimd.dma_start(outr[:, b0:b0 + SB, :], xs[:])

        nsb = NBLK // SB
        prev = front(0)
        for sb in range(1, nsb):
            cur = front(sb)
            back(sb - 1, prev)
            prev = cur
        back(nsb - 1, prev)
    nc.compile()
    return nc


# --------------------------------------------------------------------------
# orchestration
# --------------------------------------------------------------------------

def _get_program(key, builder):
    if key not in _program_cache:
        _program_cache[key] = builder()
    return _program_cache[key]


def _run(stage, nc, in_maps):
    if TRACE:
        _install_profile_shim()
    res = run_bass_kernel_spmd(nc, in_maps, core_ids=list(range(NCORE)),
                               trace=TRACE, trace_cores=list(range(NCORE)),
                               stitch_traces=False)
    if res.exec_time_ns is not None:
        _last_stage_times[stage] = res.exec_time_ns
    return res


def _fold_ws(W, a, heads, out_c):
    Wr = W.reshape(W.shape[0], heads, out_c)
    return np.einsum('fhc,hc->fh', Wr.astype(np.float64),
                     a.astype(np.float64)).astype(np.float32)


def _bcast(b, outc, dtype):
    return np.ascontiguousarray(
        np.broadcast_to(np.asarray(b, np.float32).reshape(1, outc),
                        (128, outc)).astype(dtype))


def _pad_tab(tab):
    """[N, C] node table -> [NPAD+1, C] with zero pad rows (row N..: zeros)."""
    out = np.zeros((NPAD + 1, tab.shape[1]), tab.dtype)
    out[:tab.shape[0]] = tab
    return out


def kernel(x, edge_index, W1, a_src1, a_dst1, b1, W2, a_src2, a_dst2, b2,
           W3, a_src3, a_dst3, b3):
    x = np.asarray(x, np.float32)
    W1 = np.asarray(W1, np.float32); W2 = np.asarray(W2, np.float32)
    W3 = np.asarray(W3, np.float32)

    cores, cmax, slot_of_node, node_of_slot = _plan(np.asarray(edge_index))

    ws1 = _fold_ws(W1, np.asarray(a_src1, np.float32), 4, 128)
    wd1 = _fold_ws(W1, np.asarray(a_dst1, np.float32), 4, 128)
    ws2 = _fold_ws(W2, np.asarray(a_src2, np.float32), 4, 64)
    wd2 = _fold_ws(W2, np.asarray(a_dst2, np.float32), 4, 64)
    ws3 = _fold_ws(W3, np.asarray(a_src3, np.float32), 1, 2)
    wd3 = _fold_ws(W3, np.asarray(a_dst3, np.float32), 1, 2)

    # node features in slot order (xs[slot] = x[node_of_slot[slot]])
    xs = np.zeros((NPAD, 128), np.float32)
    real = node_of_slot < N
    xs[real] = x[node_of_slot[real]]

    # stage 1: as1/ad1 for all slots (shard-local rows)
    s1 = _get_program("proj1", _build_proj1)
    wsd1 = np.concatenate([ws1, wd1], axis=1)
    r1 = _run("proj1", s1, [
        {"xT": np.ascontiguousarray(xs[c * SHARD:(c + 1) * SHARD].T),
         "W": wsd1} for c in range(NCORE)])
    asad1 = np.concatenate([r1.results[c]["out"].T for c in range(NCORE)])
    # as1 by node id: asad1 is slot-indexed
    as1_node = np.zeros((NPAD + 1, 4), np.float32)
    as1_node[node_of_slot] = asad1[:, 0:4]
    ad1_slot = asad1[:, 4:8]

    # stage 2: agg1 + W1 proj -> x2raw (slot-major [NPAD, 512] bf16)
    xtab = np.zeros((NPAD + 1, 128), BF)       # node-indexed feature table
    xtab[:N] = x.astype(BF)
    s2 = _get_program(("agg1", cmax), lambda: _build_agg1(cmax))
    in2 = [{"fm": _fm_stream(cores[c], xtab),
            "we": _we_stream(cores[c], as1_node, ad1_slot, c, cmax),
            "W1b": W1.astype(BF),
            "bt1": _bcast(b1, 512, BF)} for c in range(NCORE)]
    r2 = _run("agg1", s2, in2)
    x2 = np.concatenate([r2.results[c]["out"] for c in range(NCORE)])

    # stage 3: proj2 (elu + W2|ws2|wd2)
    s3 = _get_program("proj2", _build_proj2)
    W2s = np.concatenate([W2, ws2, wd2], axis=1).astype(BF)
    r3 = _run("proj2", s3, [
        {"xT": np.ascontiguousarray(x2[c * SHARD:(c + 1) * SHARD].T),
         "W": W2s} for c in range(NCORE)])
    h2 = np.concatenate([r3.results[c]["oh"] for c in range(NCORE)])
    sc2 = np.concatenate([r3.results[c]["osc"] for c in range(NCORE)])
    as2_node = np.zeros((NPAD + 1, 4), np.float32)
    as2_node[node_of_slot] = sc2[:, 0:4]
    ad2_slot = sc2[:, 4:8]

    # stage 4: agg2 -> x3raw
    h2tab = np.zeros((NPAD + 1, 256), BF)
    h2tab[node_of_slot] = h2
    s4 = _get_program(("agg2", cmax), lambda: _build_agg2(cmax))
    in4 = [{"fm": _fm_stream(cores[c], h2tab),
            "we": _we_stream(cores[c], as2_node, ad2_slot, c, cmax),
            "bt2": _bcast(b2, 256, BF)} for c in range(NCORE)]
    r4 = _run("agg2", s4, in4)
    x3 = np.concatenate([r4.results[c]["out"] for c in range(NCORE)])

    # stage 5: proj3 (elu + W3|ws3|wd3), W stationary, transposed out
    s5 = _get_program("proj3", _build_proj3)
    W3s = np.concatenate([W3, ws3, wd3], axis=1).astype(BF)
    r5 = _run("proj3", s5, [
        {"xT": np.ascontiguousarray(x3[c * SHARD:(c + 1) * SHARD].T),
         "W": W3s} for c in range(NCORE)])
    h3sc = np.concatenate([r5.results[c]["out"].T for c in range(NCORE)])
    h3_slot, as3_slot, ad3_slot = (h3sc[:, 0:2], h3sc[:, 2:3], h3sc[:, 3:4])
    h3tab = np.zeros((NPAD + 1, 2), BF)
    h3tab[node_of_slot] = h3_slot
    as3_node = np.zeros((NPAD + 1, 1), np.float32)
    as3_node[node_of_slot] = as3_slot

    # stage 6: agg3 -> out
    s6 = _get_program(("agg3", cmax), lambda: _build_agg3(cmax))
    in6 = [{"fm": _fm_stream(cores[c], h3tab, pad=2),
            "we": _we_stream(cores[c], as3_node, ad3_slot, c, cmax),
            "bt3": _bcast(b3, 2, np.float32)} for c in range(NCORE)]
    r6 = _run("agg3", s6, in6)
    outp = np.concatenate([r6.results[c]["out"] for c in range(NCORE)])
    return np.ascontiguousarray(outp[slot_of_node[:N]]).astype(np.float32)


# revision 30
# speedup vs baseline: 4.4939x; 1.0075x over previous
"""3-layer GAT (PyG GATConv-style) on 8 Trainium2 NeuronCores.

Strategy (dst-node sharding, all fp math on device):
  - Nodes are permuted into 100352 slots = 8 shards x 98 blocks x 128 via an
    LPT bin-pack on in-degree so every block has ~equal incoming-edge count
    (cmax = ceil(max_block_edges/128) drops 6 -> 5).
  - Edges (incl. self-loops) are partitioned by dst block; per (core, block)
    the edge list is padded to cmax 128-edge chunks; all 8 cores run one SPMD
    program per stage.
  - Segment softmax + weighted segment-sum run on TensorE via HOST-BUILT
    one-hot dst masks, DMA'd in one interleaved [features | mask] stream per
    2 blocks (no device gather, few big DMAs — the SP sequencer serializes
    dma_start issue at ~0.6us each, so DMA count matters).  The softmax
    max-shift is dropped (shift invariance; logits are O(10), safe in fp32
    exp range); the denominator comes from a mask.T @ w matmul and is
    applied at PSUM evacuation.  Self-loops guarantee den > 0 (no epsilon).
  - Per-head exp-weight scaling (wm/fs) runs as one broadcast TENSOR_TENSOR
    on VectorE for most chunks + Copy-scale ACTIVATEs on ScalarE for the
    rest (GPSIMD elementwise is ~2us/op — never used).  Each agg stage is
    software-pipelined: block i's PSUM-side work (recip/evac/proj/store) is
    emitted after block i+1's stream work to avoid head-of-line blocking on
    the strict-FIFO engine queues.  Output DMAs go via the idle GpSimd
    SWDGE ring to keep the SP sequencer free.
  - 6 SPMD stages: proj1 (as/ad), agg1(+W1 proj), proj2 (elu+W2|as2|ad2),
    agg2, proj3 (elu+W3|as3|ad3), agg3.  elu is applied in the proj stage
    that consumes each raw aggregate.  Host glue does layout only
    (permutation, row gathers by edge index, transpose, pad).
"""
import sys

sys.path.insert(0, "/opt/trn_rl_repo")

import heapq
import numpy as np
import ml_dtypes
from contextlib import ExitStack

import concourse.bass as bass
import concourse.bacc as bacc
import concourse.tile as tile
import concourse.mybir as mybir
from concourse.bass_utils import run_bass_kernel_spmd

F32 = mybir.dt.float32
F32R = mybir.dt.float32r
BF16 = mybir.dt.bfloat16
AL = mybir.AluOpType
AF = mybir.ActivationFunctionType
BF = ml_dtypes.bfloat16

N = 100000
NPAD = 100352            # 8 * 98 * 128
NCORE = 8
SHARD = NPAD // NCORE    # 12544
NBLK = SHARD // 128      # 98
NBLK_ALL = NPAD // 128   # 784
NEG = 0.2

_program_cache = {}
_last_stage_times = {}   # stage -> exec_time_ns (filled when trace=True)
TRACE = False
_shim_done = [False]


def _install_profile_shim():
    """The agent image's antenv lacks axon_hooks; recreate the tiny shim so
    run_bass_kernel_spmd(trace=True) can drive NTFF profiling via the axon
    plugin's C ABI, and stub the S3 artifact upload (no creds here)."""
    if _shim_done[0]:
        return
    import types
    mod = types.ModuleType("antenv.axon_hooks")
    holder = [None]
    mod.set_axon_ntff_profile_hook = lambda h: holder.__setitem__(0, h)
    mod.get_axon_ntff_profile_hook = lambda: holder[0]
    sys.modules["antenv.axon_hooks"] = mod
    from trn_agent_boot.trn_boot import _ntff_profile_via_ctypes
    holder[0] = _ntff_profile_via_ctypes('/opt/axon/libaxon_pjrt.so')
    import concourse.bass_utils as bu
    bu.upload_artifacts = lambda tmpdir: "local://" + str(tmpdir)
    _shim_done[0] = True


# --------------------------------------------------------------------------
# host-side preprocessing (indices / layout only)
# --------------------------------------------------------------------------

def _balance_slots(deg):
    """LPT bin-pack: assign nodes (by desc in-degree) to 784 blocks of 128
    slots so block edge counts are ~equal.  Returns slot_of_node[NPAD]."""
    order = np.argsort(-deg, kind="stable")
    heap = [(0, 0, b) for b in range(NBLK_ALL)]
    heapq.heapify(heap)
    slot_of_node = np.empty(NPAD, np.int64)
    fill = np.zeros(NBLK_ALL, np.int32)
    # process in runs of equal degree: round-robin via heap
    for n in order:
        s, c, b = heapq.heappop(heap)
        slot_of_node[n] = b * 128 + fill[b]
        fill[b] += 1
        c += 1
        if c < 128:
            heapq.heappush(heap, (s + int(deg[n]), c, b))
    return slot_of_node


def _plan(edge_index):
    src = np.concatenate([np.asarray(edge_index[0], np.int64),
                          np.arange(N, dtype=np.int64)])
    dst = np.concatenate([np.asarray(edge_index[1], np.int64),
                          np.arange(N, dtype=np.int64)])
    deg = np.bincount(dst, minlength=NPAD)  # padded "nodes" N..NPAD-1: deg 0
    slot_of_node = _balance_slots(deg)
    node_of_slot = np.empty(NPAD, np.int64)
    node_of_slot[slot_of_node] = np.arange(NPAD)

    dslot = slot_of_node[dst]
    blk = dslot // 128
    order = np.argsort(blk * 256 + (dslot % 128) // 64, kind="stable")
    src, dslot, blk = src[order], dslot[order], blk[order]
    bc = np.bincount(blk, minlength=NBLK_ALL)
    cmax = int(np.ceil(bc.max() / 128))
    starts = np.zeros(NBLK_ALL + 1, np.int64)
    np.cumsum(bc, out=starts[1:])
    L = cmax * 128

    # token slot for each edge: block-local position + block base
    within = np.arange(len(src)) - starts[blk]
    tok = blk * L + within                      # global padded token index
    T_all = NBLK_ALL * L
    tok_src = np.full(T_all, N, np.int64)       # N -> zero row in tables
    tok_dstl = np.full(T_all, 0, np.int64)
    tok_valid = np.zeros(T_all, bool)
    tok_src[tok] = src
    tok_dstl[tok] = dslot % 128
    tok_valid[tok] = True

    Tc = NBLK * L                               # tokens per core
    cores = []
    for c in range(NCORE):
        sl = slice(c * Tc, (c + 1) * Tc)
        cores.append(dict(src=tok_src[sl], dstl=tok_dstl[sl],
                          valid=tok_valid[sl]))
    return cores, cmax, slot_of_node, node_of_slot


def _chunkmaj(rows):
    """[T, C] row-major -> [128, T/128 * C] token-partition-major."""
    Tn, C = rows.shape
    ch = rows.reshape(Tn // 128, 128, C).transpose(1, 0, 2).reshape(128, -1)
    return np.ascontiguousarray(ch)


def _stream(core_plan, table, dtype):
    """Host row-gather by token src id -> [128, T/128 * C]."""
    Tn = core_plan["src"].shape[0]
    C = table.shape[1]
    rows = np.zeros((Tn, C), dtype)
    v = core_plan["valid"]
    rows[v] = table[core_plan["src"][v]].astype(dtype)
    return _chunkmaj(rows)


def _fm_stream(core_plan, table, pad=0):
    """Host [gathered-features | one-hot mask | pad] -> [128, T/128*W] bf16."""
    Tn = core_plan["src"].shape[0]
    C = table.shape[1]
    rows = np.zeros((Tn, C + 128 + pad), BF)
    v = np.nonzero(core_plan["valid"])[0]
    rows[v, 0:C] = table[core_plan["src"][v]]
    rows[v, C + core_plan["dstl"][v]] = 1
    return _chunkmaj(rows)


def _we_stream(core_plan, src_tab, dst_tab, core_id, cmax):
    """[as(src) | ad(dst)] per token -> [128, T/128 * 2H] f32.
    src_tab/dst_tab are [NPAD(+1), H] node-indexed (row N.. = zeros)."""
    Tn = core_plan["src"].shape[0]
    H = src_tab.shape[1]
    rows = np.zeros((Tn, 2 * H), np.float32)
    v = core_plan["valid"]
    rows[v, :H] = src_tab[core_plan["src"][v]]
    L = cmax * 128
    blkl = np.arange(Tn) // L
    dst_slot = (core_id * NBLK + blkl) * 128 + core_plan["dstl"]
    rows[v, H:] = dst_tab[dst_slot[v]]          # dst_tab slot-indexed
    return _chunkmaj(rows)


# --------------------------------------------------------------------------
# stage program builders
# --------------------------------------------------------------------------

def _build_proj1():
    """asadT[8, SHARD] = ([ws1|wd1].T @ xT) with W stationary."""
    GB = 7                                   # blocks per group (98 = 14 x 7)
    ngrp = NBLK // GB
    nc = bacc.Bacc("TRN2", target_bir_lowering=False, debug=False,
                   num_devices=NCORE)
    xT = nc.dram_tensor("xT", [128, SHARD], F32, kind="ExternalInput").ap()
    W = nc.dram_tensor("W", [128, 8], F32, kind="ExternalInput").ap()
    out = nc.dram_tensor("out", [8, SHARD], F32, kind="ExternalOutput").ap()

    with tile.TileContext(nc) as tc, ExitStack() as ctx:
        cpool = ctx.enter_context(tc.tile_pool(name="c", bufs=1))
        pool = ctx.enter_context(tc.tile_pool(name="p", bufs=4))
        epool = ctx.enter_context(tc.tile_pool(name="e", bufs=4))
        psum = ctx.enter_context(tc.tile_pool(name="ps", bufs=3, space="PSUM"))
        wt = cpool.tile([128, 8], F32)
        nc.sync.dma_start(wt[:], W[:])
        xTr = xT.rearrange("p (b n) -> p b n", n=128)
        for g in range(ngrp):
            xt = pool.tile([128, GB, 128], F32, tag="xt")
            nc.sync.dma_start(xt[:], xTr[:, g * GB:(g + 1) * GB, :])
            ps = psum.tile([8, GB * 128], F32, tag="ps")
            for j in range(GB):
                nc.tensor.matmul(ps[:, j * 128:(j + 1) * 128],
                                 wt[:], xt[:, j, :],
                                 start=True, stop=True)
            ot = epool.tile([8, GB * 128], F32, tag="ot")
            nc.vector.tensor_copy(ot[:], ps[:])
            nc.gpsimd.dma_start(out[:, g * GB * 128:(g + 1) * GB * 128],
                                ot[:])
    nc.compile()
    return nc


def _build_agg1(cmax):
    """agg1 + W1 proj: x2raw[dst, 512] = (sum alpha x_src) @ W1 + b1
    (no elu here; proj2 applies it).  Software-pipelined: heavy PSUM-side
    work of block i is emitted after the stream work of block i+1."""
    CM = cmax
    WEB = 4                                   # blocks per we-DMA batch
    SB = 2                                    # blocks per stream-DMA batch
    NACT = max(1, CM // 4)                    # wm chunks built on ScalarE
    NDVE = CM - NACT                          # wm chunks built on VectorE
    W = 256                                   # stream width: fe 128 | mask 128
    nc = bacc.Bacc("TRN2", target_bir_lowering=False, debug=False,
                   num_devices=NCORE)
    fm = nc.dram_tensor("fm", [128, NBLK * CM * W], BF16,
                        kind="ExternalInput").ap()
    we = nc.dram_tensor("we", [128, NBLK * CM * 8], F32,
                        kind="ExternalInput").ap()
    W1b = nc.dram_tensor("W1b", [128, 512], BF16, kind="ExternalInput").ap()
    bt1 = nc.dram_tensor("bt1", [128, 512], BF16, kind="ExternalInput").ap()
    out = nc.dram_tensor("out", [SHARD, 512], BF16, kind="ExternalOutput").ap()

    with tile.TileContext(nc) as tc, ExitStack() as ctx:
        cpool = ctx.enter_context(tc.tile_pool(name="c", bufs=1))
        spool = ctx.enter_context(tc.tile_pool(name="s", bufs=4))
        vpool = ctx.enter_context(tc.tile_pool(name="v", bufs=3))
        wpool = ctx.enter_context(tc.tile_pool(name="w", bufs=4))
        epool = ctx.enter_context(tc.tile_pool(name="e", bufs=3))
        opool = ctx.enter_context(tc.tile_pool(name="o", bufs=3))
        pagg = ctx.enter_context(tc.tile_pool(name="pa", bufs=3, space="PSUM"))
        pden = ctx.enter_context(tc.tile_pool(name="pd", bufs=2, space="PSUM"))
        po1 = ctx.enter_context(tc.tile_pool(name="po", bufs=3, space="PSUM"))

        w1t = cpool.tile([128, 512], BF16)
        nc.sync.dma_start(w1t[:], W1b[:])
        btt = cpool.tile([128, 512], BF16)
        nc.sync.dma_start(btt[:], bt1[:])

        state = {}

        def front(bi):
            c0 = bi * CM
            if bi % WEB == 0:
                nw = min(WEB, NBLK - bi) * CM
                wet = vpool.tile([128, WEB * CM, 8], F32, tag="wet")
                nc.sync.dma_start(wet[:, 0:nw, :], we[:, c0 * 8:(c0 + nw) * 8])
                zt = vpool.tile([128, WEB * CM, 4], F32, tag="zt")
                nc.vector.tensor_add(zt[:, 0:nw, :], wet[:, 0:nw, 0:4],
                                     wet[:, 0:nw, 4:8])
                nc.vector.scalar_tensor_tensor(zt[:, 0:nw, :], zt[:, 0:nw, :],
                                               NEG, zt[:, 0:nw, :],
                                               AL.mult, AL.max)
                wf = vpool.tile([128, WEB * CM, 4], F32, tag="wf")
                nc.scalar.activation(wf[:, 0:nw, :], zt[:, 0:nw, :], AF.Exp)
                wb = vpool.tile([128, WEB * CM, 4], BF16, tag="wb")
                nc.vector.tensor_copy(wb[:, 0:nw, :], wf[:, 0:nw, :])
                state["wf"], state["wb"] = wf, wb
            if bi % SB == 0:
                ns = min(SB, NBLK - bi) * CM
                st = spool.tile([128, SB * CM, W], BF16, tag="st")
                nc.sync.dma_start(st[:, 0:ns, :], fm[:, c0 * W:(c0 + ns) * W])
                state["st"] = st
            wf, wb = state["wf"], state["wb"]
            st = state["st"]
            q = (bi % WEB) * CM
            s = (bi % SB) * CM
            fet = st[:, s:s + CM, 0:128]
            mkt = st[:, s:s + CM, 128:256]

            wm = wpool.tile([128, CM, 4, 128], BF16, tag="wm")
            nc.vector.tensor_tensor(
                wm[:, 0:NDVE, :, :],
                st[:, s:s + NDVE, 128:256].unsqueeze(2).broadcast_to(
                    [128, NDVE, 4, 128]),
                wf[:, q:q + NDVE, :].unsqueeze(3).broadcast_to(
                    [128, NDVE, 4, 128]),
                AL.mult)
            for ci in range(NDVE, CM):
                for h in range(4):
                    nc.scalar.activation(wm[:, ci, h, :],
                                         st[:, s + ci, 128:256],
                                         AF.Copy, scale=wf[:, q + ci, h:h + 1])

            aggT = pagg.tile([128, 512], F32, tag="aggT")
            den = pden.tile([128, 4], F32, tag="den")
            for ci in range(CM):
                nc.tensor.matmul(
                    aggT[:], fet[:, ci, :],
                    wm[:, ci, :, :].rearrange("p h d -> p (h d)"),
                    start=(ci == 0), stop=(ci == CM - 1))
            for ci in range(CM):
                nc.tensor.matmul(den[:], mkt[:, ci, :], wb[:, q + ci, :],
                                 start=(ci == 0), stop=(ci == CM - 1),
                                 skip_group_check=True)
            return aggT, den

        def back(bi, aggT, den):
            recip = epool.tile([128, 4], F32, tag="recip")
            nc.vector.reciprocal(recip[:], den[:])
            aggTs = epool.tile([128, 512], BF16, tag="aggTs")
            nc.vector.tensor_copy(aggTs[:, 0:256], aggT[:, 0:256])
            nc.scalar.activation(aggTs[:, 256:512], aggT[:, 256:512], AF.Copy)
            o1 = po1.tile([128, 512], F32, tag="o1")
            for h in range(4):
                sl = slice(h * 128, (h + 1) * 128)
                nc.tensor.matmul(o1[:, sl], aggTs[:, sl], w1t[:, sl],
                                 start=True, stop=True)
            if bi % SB == 0:
                state["xo"] = opool.tile([128, SB, 512], BF16, tag="xo", name="xo")
            xo = state["xo"][:, bi % SB, :]
            for h in range(2):
                sl = slice(h * 128, (h + 1) * 128)
                nc.vector.scalar_tensor_tensor(
                    xo[:, sl], o1[:, sl], recip[:, h:h + 1], btt[:, sl],
                    AL.mult, AL.add)
            for h in range(2, 4):
                sl = slice(h * 128, (h + 1) * 128)
                nc.scalar.activation(xo[:, sl], o1[:, sl], AF.Copy,
                                     scale=recip[:, h:h + 1])
            nc.vector.tensor_add(xo[:, 256:512], xo[:, 256:512],
                                 btt[:, 256:512])
            if bi % SB == SB - 1 or bi == NBLK - 1:
                b0 = bi - bi % SB
                nb = bi % SB + 1
                nc.gpsimd.dma_start(
                    out.rearrange("(b p) c -> p b c", p=128)[:, b0:b0 + nb, :],
                    state["xo"][:, 0:nb, :])

        prev = front(0)
        for bi in range(1, NBLK):
            cur = front(bi)
            back(bi - 1, *prev)
            prev = cur
        back(NBLK - 1, *prev)
    nc.compile()
    return nc


def _build_proj2():
    """h2sc = elu(x2raw).T-proj: in x2T [512, SHARD] bf16;
    out h2 [SHARD, 256] bf16 + sc [SHARD, 8] f32 (as2|ad2)."""
    nc = bacc.Bacc("TRN2", target_bir_lowering=False, debug=False,
                   num_devices=NCORE)
    xT = nc.dram_tensor("xT", [512, SHARD], BF16, kind="ExternalInput").ap()
    W = nc.dram_tensor("W", [512, 264], BF16, kind="ExternalInput").ap()
    oh = nc.dram_tensor("oh", [SHARD, 256], BF16, kind="ExternalOutput").ap()
    osc = nc.dram_tensor("osc", [SHARD, 8], F32, kind="ExternalOutput").ap()

    with tile.TileContext(nc) as tc, ExitStack() as ctx:
        cpool = ctx.enter_context(tc.tile_pool(name="c", bufs=1))
        pool = ctx.enter_context(tc.tile_pool(name="p", bufs=4))
        epool = ctx.enter_context(tc.tile_pool(name="e", bufs=4))
        psum = ctx.enter_context(tc.tile_pool(name="ps", bufs=4, space="PSUM"))
        SB = 2
        GB = 8                                   # blocks of sc staged per DMA
        wt = cpool.tile([128, 4, 264], BF16)
        for k in range(4):
            nc.sync.dma_start(wt[:, k, :], W[128 * k:128 * (k + 1), :])
        xTr = xT.rearrange("(k p) n -> p k n", p=128)
        state = {}

        def front(bi):
            if bi % SB == 0:
                n = min(SB, NBLK - bi) * 128
                xt = pool.tile([128, 4, SB * 128], BF16, tag="xt")
                nc.sync.dma_start(xt[:, :, 0:n],
                                  xTr[:, :, 128 * bi:128 * bi + n])
                # elu in place: xe = max(exp(min(x,0))-1, x)
                mt = pool.tile([128, 4, SB * 128], BF16, tag="mt")
                nc.vector.tensor_scalar_min(mt[:, :, 0:n], xt[:, :, 0:n], 0.0)
                nc.scalar.activation(mt[:, :, 0:n], mt[:, :, 0:n], AF.Exp)
                nc.vector.scalar_tensor_tensor(xt[:, :, 0:n], mt[:, :, 0:n],
                                               1.0, xt[:, :, 0:n],
                                               AL.subtract, AL.max)
                state["xt"] = xt
            xt = state["xt"]
            s = (bi % SB) * 128
            ps = psum.tile([128, 264], F32, tag="ps")
            for k in range(4):
                nc.tensor.matmul(ps[:], xt[:, k, s:s + 128], wt[:, k, :],
                                 start=(k == 0), stop=(k == 3))
            return (ps,)

        def back(bi, ps):
            if bi % SB == 0:
                state["ht"] = epool.tile([128, SB, 256], BF16, tag="ht", name="ht")
            ht = state["ht"][:, bi % SB, :]
            nc.vector.tensor_copy(ht[:, 0:128], ps[:, 0:128])
            nc.scalar.activation(ht[:, 128:256], ps[:, 128:256], AF.Copy)
            if bi % SB == SB - 1 or bi == NBLK - 1:
                b0 = bi - bi % SB
                nb = bi % SB + 1
                nc.gpsimd.dma_start(
                    oh.rearrange("(b p) c -> p b c", p=128)[:, b0:b0 + nb, :],
                    state["ht"][:, 0:nb, :])
            g, j = bi // GB, bi % GB
            if j == 0:
                state["sct"] = epool.tile([128, GB, 8], F32, tag="sct", name="sct")
            nc.vector.tensor_copy(state["sct"][:, j, :], ps[:, 256:264])
            if j == GB - 1 or bi == NBLK - 1:
                nb = j + 1
                oscr = osc.rearrange("(b p) c -> p b c", p=128)
                nc.gpsimd.dma_start(oscr[:, g * GB:g * GB + nb, :],
                                    state["sct"][:, 0:nb, :])

        prev = front(0)
        for bi in range(1, NBLK):
            cur = front(bi)
            back(bi - 1, *prev)
            prev = cur
        back(NBLK - 1, *prev)
    nc.compile()
    return nc


def _build_agg2(cmax):
    """agg2: x3raw[dst, 256] = (sum alpha h2_src) + b2 (no elu; proj3)."""
    CM = cmax
    W = 384                                   # fe 256 | mask 128
    nc = bacc.Bacc("TRN2", target_bir_lowering=False, debug=False,
                   num_devices=NCORE)
    fm = nc.dram_tensor("fm", [128, NBLK * CM * W], BF16,
                        kind="ExternalInput").ap()
    we = nc.dram_tensor("we", [128, NBLK * CM * 8], F32,
                        kind="ExternalInput").ap()
    bt2 = nc.dram_tensor("bt2", [128, 256], BF16, kind="ExternalInput").ap()
    out = nc.dram_tensor("out", [SHARD, 256], BF16, kind="ExternalOutput").ap()

    with tile.TileContext(nc) as tc, ExitStack() as ctx:
        cpool = ctx.enter_context(tc.tile_pool(name="c", bufs=1))
        spool = ctx.enter_context(tc.tile_pool(name="s", bufs=4))
        vpool = ctx.enter_context(tc.tile_pool(name="v", bufs=3))
        wpool = ctx.enter_context(tc.tile_pool(name="w", bufs=4))
        epool = ctx.enter_context(tc.tile_pool(name="e", bufs=3))
        opool = ctx.enter_context(tc.tile_pool(name="o", bufs=3))
        pagg = ctx.enter_context(tc.tile_pool(name="pa", bufs=4, space="PSUM"))

        WEB = 4
        SB = 2
        NACT = 1                              # fs chunks built on ScalarE
        NDVE = CM - NACT
        btt = cpool.tile([128, 256], BF16)
        nc.sync.dma_start(btt[:], bt2[:])

        state = {}

        def front(bi):
            c0 = bi * CM
            if bi % WEB == 0:
                nw = min(WEB, NBLK - bi) * CM
                wet = vpool.tile([128, WEB * CM, 8], F32, tag="wet")
                nc.sync.dma_start(wet[:, 0:nw, :], we[:, c0 * 8:(c0 + nw) * 8])
                zt = vpool.tile([128, WEB * CM, 4], F32, tag="zt")
                nc.vector.tensor_add(zt[:, 0:nw, :], wet[:, 0:nw, 0:4],
                                     wet[:, 0:nw, 4:8])
                nc.vector.scalar_tensor_tensor(zt[:, 0:nw, :], zt[:, 0:nw, :],
                                               NEG, zt[:, 0:nw, :],
                                               AL.mult, AL.max)
                wf = vpool.tile([128, WEB * CM, 4], F32, tag="wf")
                nc.scalar.activation(wf[:, 0:nw, :], zt[:, 0:nw, :], AF.Exp)
                state["wf"] = wf
            if bi % SB == 0:
                ns = min(SB, NBLK - bi) * CM
                st = spool.tile([128, SB * CM, W], BF16, tag="st")
                nc.sync.dma_start(st[:, 0:ns, :], fm[:, c0 * W:(c0 + ns) * W])
                state["st"] = st
            wf = state["wf"]
            st = state["st"]
            q = (bi % WEB) * CM
            s = (bi % SB) * CM
            mkt = st[:, s:s + CM, 256:384]
            # fs laid out [128, CM, 264]: cols 0:256 scaled features,
            # cols 256:260 = w (bf16) so one rhs serves agg+den.
            fs = wpool.tile([128, CM, 264], BF16, tag="fs")
            nc.vector.tensor_copy(fs[:, :, 256:260], wf[:, q:q + CM, :])
            fsv = fs[:, 0:NDVE, 0:256].rearrange("p c (h f) -> p c h f", h=4)
            nc.vector.tensor_tensor(
                fsv,
                st[:, s:s + NDVE, 0:256].rearrange("p c (h f) -> p c h f",
                                                   h=4),
                wf[:, q:q + NDVE, :].unsqueeze(3).broadcast_to(
                    [128, NDVE, 4, 64]),
                AL.mult)
            for ci in range(NDVE, CM):
                for h in range(4):
                    nc.scalar.activation(fs[:, ci, h * 64:(h + 1) * 64],
                                         st[:, s + ci, h * 64:(h + 1) * 64],
                                         AF.Copy, scale=wf[:, q + ci, h:h + 1])

            agg = pagg.tile([128, 260], F32, tag="agg")
            for ci in range(CM):
                nc.tensor.matmul(agg[:], mkt[:, ci, :], fs[:, ci, 0:260],
                                 start=(ci == 0), stop=(ci == CM - 1))
            return (agg,)

        def back(bi, agg):
            recip = epool.tile([128, 4], F32, tag="recip")
            nc.vector.reciprocal(recip[:], agg[:, 256:260])
            if bi % SB == 0:
                state["xo"] = opool.tile([128, SB, 256], BF16, tag="xo", name="xo")
            xo = state["xo"][:, bi % SB, :]
            for h in range(2):
                sl = slice(h * 64, (h + 1) * 64)
                nc.vector.scalar_tensor_tensor(
                    xo[:, sl], agg[:, sl], recip[:, h:h + 1], btt[:, sl],
                    AL.mult, AL.add)
            for h in range(2, 4):
                sl = slice(h * 64, (h + 1) * 64)
                nc.scalar.activation(xo[:, sl], agg[:, sl], AF.Copy,
                                     scale=recip[:, h:h + 1])
            nc.vector.tensor_add(xo[:, 128:256], xo[:, 128:256],
                                 btt[:, 128:256])
            if bi % SB == SB - 1 or bi == NBLK - 1:
                b0 = bi - bi % SB
                nb = bi % SB + 1
                nc.gpsimd.dma_start(
                    out.rearrange("(b p) c -> p b c", p=128)[:, b0:b0 + nb, :],
                    state["xo"][:, 0:nb, :])

        prev = front(0)
        for bi in range(1, NBLK):
            cur = front(bi)
            back(bi - 1, *prev)
            prev = cur
        back(NBLK - 1, *prev)
    nc.compile()
    return nc


def _build_proj3():
    """h3scT = [W3|ws3|wd3].T @ elu(x3raw).T: in x3T [256, SHARD] bf16;
    out [4, SHARD] f32 (h3 2 | as3 | ad3), W stationary."""
    GB = 7                                   # 98 = 14 x 7
    ngrp = NBLK // GB
    nc = bacc.Bacc("TRN2", target_bir_lowering=False, debug=False,
                   num_devices=NCORE)
    xT = nc.dram_tensor("xT", [256, SHARD], BF16, kind="ExternalInput").ap()
    W = nc.dram_tensor("W", [256, 4], BF16, kind="ExternalInput").ap()
    out = nc.dram_tensor("out", [4, SHARD], F32, kind="ExternalOutput").ap()

    with tile.TileContext(nc) as tc, ExitStack() as ctx:
        cpool = ctx.enter_context(tc.tile_pool(name="c", bufs=1))
        pool = ctx.enter_context(tc.tile_pool(name="p", bufs=4))
        epool = ctx.enter_context(tc.tile_pool(name="e", bufs=4))
        psum = ctx.enter_context(tc.tile_pool(name="ps", bufs=3, space="PSUM"))
        wt = cpool.tile([128, 2, 4], BF16)
        for k in range(2):
            nc.sync.dma_start(wt[:, k, :], W[128 * k:128 * (k + 1), :])
        xTr = xT.rearrange("(k p) n -> p k n", p=128)
        for g in range(ngrp):
            xt = pool.tile([128, 2, GB, 128], BF16, tag="xt")
            nc.sync.dma_start(xt[:],
                              xTr[:, :, g * GB * 128:(g + 1) * GB * 128]
                              .rearrange("p k (b n) -> p k b n", n=128))
            mt = pool.tile([128, 2, GB, 128], BF16, tag="mt")
            nc.vector.tensor_scalar_min(mt[:], xt[:], 0.0)
            nc.scalar.activation(mt[:], mt[:], AF.Exp)
            nc.vector.scalar_tensor_tensor(xt[:], mt[:], 1.0, xt[:],
                                           AL.subtract, AL.max)
            ps = psum.tile([4, GB * 128], F32, tag="ps")
            for j in range(GB):
                for k in range(2):
                    nc.tensor.matmul(ps[:, j * 128:(j + 1) * 128],
                                     wt[:, k, :], xt[:, k, j, :],
                                     start=(k == 0), stop=(k == 1))
            ot = epool.tile([4, GB * 128], F32, tag="ot")
            nc.vector.tensor_copy(ot[:], ps[:])
            nc.gpsimd.dma_start(out[:, g * GB * 128:(g + 1) * GB * 128],
                                ot[:])
    nc.compile()
    return nc


def _build_agg3(cmax):
    """agg3: out[dst, 2] = (sum alpha h3_src) + b3, heads=1."""
    CM = cmax
    SB = 7                                    # 98 = 14 x 7
    W = 132                                   # fe 2 | mask 128 | pad 2
    nc = bacc.Bacc("TRN2", target_bir_lowering=False, debug=False,
                   num_devices=NCORE)
    fm = nc.dram_tensor("fm", [128, NBLK * CM * W], BF16,
                        kind="ExternalInput").ap()
    we = nc.dram_tensor("we", [128, NBLK * CM * 2], F32,
                        kind="ExternalInput").ap()
    bt3 = nc.dram_tensor("bt3", [128, 2], F32, kind="ExternalInput").ap()
    out = nc.dram_tensor("out", [SHARD, 2], F32, kind="ExternalOutput").ap()

    with tile.TileContext(nc) as tc, ExitStack() as ctx:
        cpool = ctx.enter_context(tc.tile_pool(name="c", bufs=1))
        spool = ctx.enter_context(tc.tile_pool(name="s", bufs=4))
        wpool = ctx.enter_context(tc.tile_pool(name="w", bufs=4))
        epool = ctx.enter_context(tc.tile_pool(name="e", bufs=4))
        pagg = ctx.enter_context(tc.tile_pool(name="pa", bufs=3, space="PSUM"))

        btt = cpool.tile([128, 2], F32)
        nc.sync.dma_start(btt[:], bt3[:])

        state = {}

        def front(sb):
            b0 = sb * SB
            c0 = b0 * CM
            nch = SB * CM
            st = spool.tile([128, SB * CM, W], BF16, tag="st")
            nc.sync.dma_start(st[:], fm[:, c0 * W:(c0 + nch) * W])
            wet = spool.tile([128, SB * CM, 2], F32, tag="wet")
            nc.scalar.dma_start(wet[:], we[:, c0 * 2:(c0 + nch) * 2])

            zt = wpool.tile([128, SB * CM, 1], F32, tag="zt")
            nc.vector.tensor_add(zt[:], wet[:, :, 0:1], wet[:, :, 1:2])
            nc.vector.scalar_tensor_tensor(zt[:], zt[:], NEG, zt[:],
                                           AL.mult, AL.max)
            wf = wpool.tile([128, SB * CM, 1], F32, tag="wf")
            nc.scalar.activation(wf[:], zt[:], AF.Exp)
            rhs = wpool.tile([128, SB * CM, 4], BF16, tag="rhs")
            nc.vector.tensor_tensor(
                rhs[:, :, 0:2], st[:, :, 0:2],
                wf[:].broadcast_to([128, nch, 2]), AL.mult)
            nc.vector.tensor_copy(rhs[:, :, 2:3], wf[:])

            agg = pagg.tile([128, SB, 3], F32, tag="agg")
            for b in range(SB):
                for ci in range(CM):
                    cc = b * CM + ci
                    nc.tensor.matmul(agg[:, b, :], st[:, cc, 2:130],
                                     rhs[:, cc, 0:3],
                                     start=(ci == 0), stop=(ci == CM - 1),
                                     skip_group_check=(b > 0))
            return agg

        def back(sb, agg):
            b0 = sb * SB
            xs = epool.tile([128, SB, 2], F32, tag="xs")
            recip = epool.tile([128, SB, 1], F32, tag="recip")
            nc.vector.reciprocal(recip[:], agg[:, :, 2:3])
            for b in range(SB):
                nc.vector.scalar_tensor_tensor(
                    xs[:, b, :], agg[:, b, 0:2], recip[:, b, :], btt[:],
                    AL.mult, AL.add)
            outr = out.rearrange("(b p) c -> p b c", p=128)
            nc.gpsimd.dma_start(outr[:, b0:b0 + SB, :], xs[:])

        nsb = NBLK // SB
        prev = front(0)
        for sb in range(1, nsb):
            cur = front(sb)
            back(sb - 1, prev)
            prev = cur
        back(nsb - 1, prev)
    nc.compile()
    return nc


# --------------------------------------------------------------------------
# orchestration
# --------------------------------------------------------------------------

def _get_program(key, builder):
    if key not in _program_cache:
        _program_cache[key] = builder()
    return _program_cache[key]


def _run(stage, nc, in_maps):
    if TRACE:
        _install_profile_shim()
    res = run_bass_kernel_spmd(nc, in_maps, core_ids=list(range(NCORE)),
                               trace=TRACE, trace_cores=list(range(NCORE)),
                               stitch_traces=False)
    if res.exec_time_ns is not None:
        _last_stage_times[stage] = res.exec_time_ns
    return res


def _fold_ws(W, a, heads, out_c):
    Wr = W.reshape(W.shape[0], heads, out_c)
    return np.einsum('fhc,hc->fh', Wr.astype(np.float64),
                     a.astype(np.float64)).astype(np.float32)


def _bcast(b, outc, dtype):
    return np.ascontiguousarray(
        np.broadcast_to(np.asarray(b, np.float32).reshape(1, outc),
                        (128, outc)).astype(dtype))


def _pad_tab(tab):
    """[N, C] node table -> [NPAD+1, C] with zero pad rows (row N..: zeros)."""
    out = np.zeros((NPAD + 1, tab.shape[1]), tab.dtype)
    out[:tab.shape[0]] = tab
    return out


def kernel(x, edge_index, W1, a_src1, a_dst1, b1, W2, a_src2, a_dst2, b2,
           W3, a_src3, a_dst3, b3):
    x = np.asarray(x, np.float32)
    W1 = np.asarray(W1, np.float32); W2 = np.asarray(W2, np.float32)
    W3 = np.asarray(W3, np.float32)

    cores, cmax, slot_of_node, node_of_slot = _plan(np.asarray(edge_index))

    ws1 = _fold_ws(W1, np.asarray(a_src1, np.float32), 4, 128)
    wd1 = _fold_ws(W1, np.asarray(a_dst1, np.float32), 4, 128)
    ws2 = _fold_ws(W2, np.asarray(a_src2, np.float32), 4, 64)
    wd2 = _fold_ws(W2, np.asarray(a_dst2, np.float32), 4, 64)
    ws3 = _fold_ws(W3, np.asarray(a_src3, np.float32), 1, 2)
    wd3 = _fold_ws(W3, np.asarray(a_dst3, np.float32), 1, 2)

    # node features in slot order (xs[slot] = x[node_of_slot[slot]])
    xs = np.zeros((NPAD, 128), np.float32)
    real = node_of_slot < N
    xs[real] = x[node_of_slot[real]]

    # stage 1: as1/ad1 for all slots (shard-local rows)
    s1 = _get_program("proj1", _build_proj1)
    wsd1 = np.concatenate([ws1, wd1], axis=1)
    r1 = _run("proj1", s1, [
        {"xT": np.ascontiguousarray(xs[c * SHARD:(c + 1) * SHARD].T),
         "W": wsd1} for c in range(NCORE)])
    asad1 = np.concatenate([r1.results[c]["out"].T for c in range(NCORE)])
    # as1 by node id: asad1 is slot-indexed
    as1_node = np.zeros((NPAD + 1, 4), np.float32)
    as1_node[node_of_slot] = asad1[:, 0:4]
    ad1_slot = asad1[:, 4:8]

    # stage 2: agg1 + W1 proj -> x2raw (slot-major [NPAD, 512] bf16)
    xtab = np.zeros((NPAD + 1, 128), BF)       # node-indexed feature table
    xtab[:N] = x.astype(BF)
    s2 = _get_program(("agg1", cmax), lambda: _build_agg1(cmax))
    in2 = [{"fm": _fm_stream(cores[c], xtab),
            "we": _we_stream(cores[c], as1_node, ad1_slot, c, cmax),
            "W1b": W1.astype(BF),
            "bt1": _bcast(b1, 512, BF)} for c in range(NCORE)]
    r2 = _run("agg1", s2, in2)
    x2 = np.concatenate([r2.results[c]["out"] for c in range(NCORE)])

    # stage 3: proj2 (elu + W2|ws2|wd2)
    s3 = _get_program("proj2", _build_proj2)
    W2s = np.concatenate([W2, ws2, wd2], axis=1).astype(BF)
    r3 = _run("proj2", s3, [
        {"xT": np.ascontiguousarray(x2[c * SHARD:(c + 1) * SHARD].T),
         "W": W2s} for c in range(NCORE)])
    h2 = np.concatenate([r3.results[c]["oh"] for c in range(NCORE)])
    sc2 = np.concatenate([r3.results[c]["osc"] for c in range(NCORE)])
    as2_node = np.zeros((NPAD + 1, 4), np.float32)
    as2_node[node_of_slot] = sc2[:, 0:4]
    ad2_slot = sc2[:, 4:8]

    # stage 4: agg2 -> x3raw
    h2tab = np.zeros((NPAD + 1, 256), BF)
    h2tab[node_of_slot] = h2
    s4 = _get_program(("agg2", cmax), lambda: _build_agg2(cmax))
    in4 = [{"fm": _fm_stream(cores[c], h2tab),
            "we": _we_stream(cores[c], as2_node, ad2_slot, c, cmax),
            "bt2": _bcast(b2, 256, BF)} for c in range(NCORE)]
    r4 = _run("agg2", s4, in4)
    x3 = np.concatenate([r4.results[c]["out"] for c in range(NCORE)])

    # stage 5: proj3 (elu + W3|ws3|wd3), W stationary, transposed out
    s5 = _get_program("proj3", _build_proj3)
    W3s = np.concatenate([W3, ws3, wd3], axis=1).astype(BF)
    r5 = _run("proj3", s5, [
        {"xT": np.ascontiguousarray(x3[c * SHARD:(c + 1) * SHARD].T),
         "W": W3s} for c in range(NCORE)])
    h3sc = np.concatenate([r5.results[c]["out"].T for c in range(NCORE)])
    h3_slot, as3_slot, ad3_slot = (h3sc[:, 0:2], h3sc[:, 2:3], h3sc[:, 3:4])
    h3tab = np.zeros((NPAD + 1, 2), BF)
    h3tab[node_of_slot] = h3_slot
    as3_node = np.zeros((NPAD + 1, 1), np.float32)
    as3_node[node_of_slot] = as3_slot

    # stage 6: agg3 -> out
    s6 = _get_program(("agg3", cmax), lambda: _build_agg3(cmax))
    in6 = [{"fm": _fm_stream(cores[c], h3tab, pad=2),
            "we": _we_stream(cores[c], as3_node, ad3_slot, c, cmax),
            "bt3": _bcast(b3, 2, np.float32)} for c in range(NCORE)]
    r6 = _run("agg3", s6, in6)
    outp = np.concatenate([r6.results[c]["out"] for c in range(NCORE)])
    return np.ascontiguousarray(outp[slot_of_node[:N]]).astype(np.float32)
